# revision 1
# baseline (speedup 1.0000x reference)
"""Trainium2 Bass kernel for nn_AMM_w_AFDM (scatter_memory).

Strategy (one batch per NeuronCore, 8 cores data-parallel):
  out[b] = feature + P + splat(P, w)  where P = nearest-cell scatter of x.
  The 11x11 splat-with-border-clipping is computed as a bank of banded
  Toeplitz matmuls on TensorE:
     ext[x, ye, c] = sum_dy  T_dy[xin, x]^T @ P[xin, ye-dy, c]
  with the x-side clip-folds absorbed into the Toeplitz border columns and
  the y-side folds applied on the extended output rows. The identity (+P)
  term is folded into the kernel center (+1).
  P is built with gpsimd.dma_scatter_add (SBUF parity-split destination).
  The AFDM dilation scalar (conv1x1 + GN + pool + MLP + sigmoid) is computed
  on-device with TensorE matmuls and DVE/ACT reductions.
"""
import sys
from contextlib import ExitStack
import numpy as np

sys.path.insert(0, "/opt/trn_rl_repo")

import concourse.bacc as bacc  # noqa: E402
import concourse.bass as bass  # noqa: E402
import concourse.mybir as mybir  # noqa: E402
import concourse.tile as tile  # noqa: E402
from concourse.ap import AP  # noqa: E402

MD = 5
EPS = 1e-5
B, N, C, H, W = 8, 4096, 64, 128, 128
F32 = mybir.dt.float32
BF16 = mybir.dt.bfloat16
I16 = mybir.dt.int16
AX = mybir.AxisListType
OP = mybir.AluOpType
AF = mybir.ActivationFunctionType

# P_T slot layout: slot t in [0,160), y = t-16 (zeros outside [0,128)).
PT_SLOTS = 160
PT_OFF = 16
# ext output slots: u in [0,144), ye = u-8.
EXT_SLOTS = 144


def build_nc():
    nc = bacc.Bacc("TRN2", target_bir_lowering=False)

    def din(name, shape, dt=F32):
        return nc.dram_tensor(name, shape, dt, kind="ExternalInput")

    x_wrap = din("x_wrap", [128, 32, 64])
    xx_w2 = din("xx_w2", [128, 32])
    xy_w2 = din("xy_w2", [128, 32])
    feat_nat2 = din("feat_nat2", [128, 8192])
    feat_T = din("feat_T", [128, 8192])
    blob1 = din("blob1", [128, 968])
    blob2 = din("blob2", [11, 546])
    w1n = din("w1n", [128, 32, 128])
    lint_dram = nc.dram_tensor("lint_dram", [32, 128], F32)

    out_T = nc.dram_tensor("out_T", [128, 8192], F32, kind="ExternalOutput")
    vdram = nc.dram_tensor("vdram", [11, 400], F32)

    with tile.TileContext(nc) as tc:
        with tc.tile_pool(name="main", bufs=1) as pool, \
             tc.tile_pool(name="scat", bufs=1) as scpool, \
             tc.tile_pool(name="stage", bufs=2) as stpool:
            psum_stack = ExitStack()

            # ============== Phase S: scatter x -> P ==============
            # Index math in the 128-wrap: lin128[p, t] for point j = t*128+p.
            ps_prep = psum_stack.enter_context(
                tc.tile_pool(name="psp", bufs=2, space="PSUM"))
            xx2 = pool.tile([128, 32], F32, tag="xx2")
            xy2 = pool.tile([128, 32], F32, tag="xy2")
            nc.sync.dma_start(out=xx2[:, :], in_=xx_w2[:, :])
            nc.sync.dma_start(out=xy2[:, :], in_=xy_w2[:, :])

            def floor127(srcw, sfx):
                # floor(t) = round(t) - (round(t) > t), round via +/- 2^23
                t = pool.tile([128, 32], F32, tag=f"fl_t{sfx}")
                r = pool.tile([128, 32], F32, tag=f"fl_r{sfx}")
                g = pool.tile([128, 32], F32, tag=f"fl_g{sfx}")
                o = pool.tile([128, 32], F32, tag=f"fl_o{sfx}")
                nc.vector.tensor_scalar_mul(t[:, :], srcw[:, :], 127.0)
                nc.vector.tensor_scalar(r[:, :], t[:, :], 8388608.0, -8388608.0,
                                        OP.add, OP.add)
                nc.vector.tensor_tensor(g[:, :], r[:, :], t[:, :], OP.is_gt)
                nc.vector.tensor_tensor(o[:, :], r[:, :], g[:, :], OP.subtract)
                return o

            xc = floor127(xx2, "x")
            yc = floor127(xy2, "y")
            lin128 = pool.tile([128, 32], F32, tag="lin128")
            nc.vector.tensor_scalar_mul(lin128[:, :], yc[:, :], 128.0)
            nc.vector.tensor_tensor(lin128[:, :], lin128[:, :], xc[:, :], OP.add)


            blob1_sb = pool.tile([128, 968], F32, tag="blob1_sb")
            nc.sync.dma_start(out=blob1_sb[:, :], in_=blob1[:, :])
            blob2_sb = pool.tile([11, 546], F32, tag="blob2_sb")
            nc.sync.dma_start(out=blob2_sb[:, :], in_=blob2[:, :])
            ltri_f = blob1_sb[:, 0:128]
            ident_v = blob1_sb[:, 128:256]
            selw_v = blob1_sb[:, 256:384]
            cw_view = blob1_sb[:, 384:896].rearrange("p (a b) -> p a b", b=128)
            trash_sb = blob1_sb[:, 896:928]
            sw_view = blob1_sb[:, 928:960].rearrange("p (a b) -> p a b", b=8)
            gm_v = blob1_sb[:, 960:964]
            cb_col = blob1_sb[:, 964:965]
            gg_col = blob1_sb[:, 965:966]
            gb_col = blob1_sb[:, 966:967]
            onesc_f = blob1_sb[:, 967:968]
            gmt_v = blob2_sb[0:4, 0:128]
            gr8_v = blob2_sb[0:8, 128:256]
            w2_v = blob2_sb[0:1, 256:384]
            o1128_v = blob2_sb[0:1, 384:512]
            dnt_v = blob2_sb[0:11, 512:523]
            cm_v = blob2_sb[0:11, 523:534]
            o11_v = blob2_sb[0:11, 534:535]
            o111_v = blob2_sb[0:1, 535:546]
            ltri_sb = pool.tile([128, 128], BF16, tag="ltri_sb")
            nc.vector.tensor_copy(ltri_sb[:, :], ltri_f)
            onesc_bf = pool.tile([128, 1], BF16, tag="onesc_bf")
            nc.vector.tensor_copy(onesc_bf[:, :], onesc_f)

            x_sb = scpool.tile([128, 32, 64], F32, tag="x_sb")
            nc.sync.dma_start(out=x_sb[:, :, :], in_=x_wrap[:, :, :])
            x_bf = scpool.tile([128, 32, 64], BF16, tag="x_bf")
            nc.scalar.activation(x_bf[:, :, :], x_sb[:, :, :], AF.Copy)
            xm = scpool.tile([128, 32, 64], BF16, tag="xm")

            # Per-256-chunk dedup: chunk a = slots (2a, 2a+1) = subs (u, v).
            # merged_u[i] = sum over all same-cell points in sub u + sub v;
            # first flags: sub u by intra-below count; sub v also requires no
            # match anywhere in sub u.
            first128 = pool.tile([128, 32], F32, tag="first128")
            ps_lt = ps_prep.tile([32, 128], F32, tag="ps_lt")
            nc.tensor.transpose(ps_lt[:, :], lin128[:, :], ident_v)
            linT = pool.tile([32, 128], F32, tag="linT")
            nc.vector.tensor_copy(linT[:, :], ps_lt[:, :])
            nc.sync.dma_start(out=lint_dram[:, :], in_=linT[:, :])

            def eq_mat(out_bf, bc_psum, bc_off, col_t):
                # out[q, p] = (lin(col_t, q) == bcast[bc_off + p])
                nc.vector.tensor_scalar(out_bf[:, :], bc_psum[:, bc_off:bc_off + 128],
                                        lin128[:, col_t:col_t + 1], None, OP.is_equal)

            for a4 in range(8):  # 4 slots (2 chunks) per bcast matmul
                lrow = stpool.tile([1, 512], F32, tag="lrow")
                nc.sync.dma_start(
                    out=lrow[:, :],
                    in_=AP(tensor=lint_dram, offset=a4 * 512, ap=[[1, 1], [1, 512]]))
                bc = ps_prep.tile([128, 512], F32, tag="bc")
                nc.tensor.matmul(bc[:, :], o1128_v, lrow[:, :],
                                 start=True, stop=True)
                for ci in range(2):
                    a = a4 * 2 + ci
                    u, v = 2 * a, 2 * a + 1
                    uoff, voff = (u % 4) * 128, (v % 4) * 128
                    m_uu = pool.tile([128, 128], BF16, tag="m_uu")
                    m_vv = pool.tile([128, 128], BF16, tag="m_vv")
                    m_uv = pool.tile([128, 128], BF16, tag="m_uv")
                    m_vu = pool.tile([128, 128], BF16, tag="m_vu")
                    eq_mat(m_uu, bc, uoff, u)   # rows q: sub u, cols p: sub u
                    eq_mat(m_vv, bc, voff, v)   # rows: v, cols: v
                    eq_mat(m_uv, bc, voff, u)   # rows: u, cols: v
                    eq_mat(m_vu, bc, uoff, v)   # rows: v, cols: u
                    # merged_u = m_uu.T @ x_u + m_vu.T @ x_v
                    pm = ps_prep.tile([128, 128], F32, tag="pm")
                    nc.tensor.matmul(pm[:, 0:64], m_uu[:, :], x_bf[:, u, :],
                                     start=True, stop=False)
                    nc.tensor.matmul(pm[:, 0:64], m_vu[:, :], x_bf[:, v, :],
                                     start=False, stop=True)
                    nc.tensor.matmul(pm[:, 64:128], m_vv[:, :], x_bf[:, v, :],
                                     start=True, stop=True)
                    nc.scalar.activation(xm[:, u:u + 2, :], pm[:, :], AF.Copy)
                    # counts: below_u = (m_uu*L).T @ 1 ; below_v = (m_vv*L).T @ 1
                    #         + m_uv.T @ 1 (any match in sub u)
                    ml_u = pool.tile([128, 128], BF16, tag="ml_u")
                    ml_v = pool.tile([128, 128], BF16, tag="ml_v")
                    nc.vector.tensor_tensor(ml_u[:, :], m_uu[:, :], ltri_sb[:, :],
                                            OP.mult)
                    nc.vector.tensor_tensor(ml_v[:, :], m_vv[:, :], ltri_sb[:, :],
                                            OP.mult)
                    cnt = ps_prep.tile([128, 2], F32, tag="cnt")
                    nc.tensor.matmul(cnt[:, 0:1], ml_u[:, :], onesc_bf[:, :],
                                     start=True, stop=True)
                    nc.tensor.matmul(cnt[:, 1:2], ml_v[:, :], onesc_bf[:, :],
                                     start=True, stop=False)
                    nc.tensor.matmul(cnt[:, 1:2], m_uv[:, :], onesc_bf[:, :],
                                     start=False, stop=True)
                    nc.vector.tensor_scalar(first128[:, u:u + 2], cnt[:, :],
                                            0.5, None, OP.is_lt)

            # idx = first ? lin : trash  (per 16-slot half, so chain A
            # scatters can start before half B dedup finishes)
            idxf = pool.tile([128, 32], F32, tag="idxf")
            idx_sb = pool.tile([128, 256], I16, tag="idx")
            idxw_f = pool.tile([16, 256], F32, tag="idxw_f")
            for hf in range(2):
                hs = slice(16 * hf, 16 * hf + 16)
                nc.vector.tensor_tensor(idxf[:, hs], lin128[:, hs],
                                        trash_sb[:, hs], OP.subtract)
                nc.vector.tensor_tensor(idxf[:, hs], idxf[:, hs],
                                        first128[:, hs], OP.mult)
                nc.vector.tensor_tensor(idxf[:, hs], idxf[:, hs],
                                        trash_sb[:, hs], OP.add)
                # 16-wrap: idxw[q, t*8+g] = idxf[16g+q, t]
                for g in range(8):
                    ps_w = ps_prep.tile([16, 16], F32, tag="bc")
                    nc.tensor.matmul(ps_w[:, :], selw_v[:, 16 * g:16 * g + 16],
                                     idxf[:, hs], start=True, stop=True)
                    dst = idxw_f[:, :]
                    nc.vector.tensor_copy(
                        AP(tensor=dst.tensor, offset=dst.offset + 128 * hf + g,
                           ap=[list(dst.ap[0]), [8, 16]]),
                        ps_w[:, :])
                nc.vector.tensor_copy(idx_sb[0:16, 128 * hf:128 * hf + 128],
                                      idxw_f[:, 128 * hf:128 * hf + 128])
            for r in range(1, 8):
                nc.sync.dma_start(out=idx_sb[16 * r:16 * r + 16, :],
                                  in_=idx_sb[0:16, :])

            # Two concurrent serialized chains (disjoint dst buffers), bf16.
            pb0 = scpool.tile([128, 66 * 64], BF16, tag="pb0")
            pb1 = scpool.tile([128, 66 * 64], BF16, tag="pb1")
            pb2 = scpool.tile([128, 66 * 64], BF16, tag="pb2")
            pb3 = scpool.tile([128, 66 * 64], BF16, tag="pb3")
            pbuf = [pb0, pb1, pb2, pb3]  # [A_even, A_odd, B_even, B_odd]
            for pb in pbuf:
                nc.gpsimd.memset(pb[:, 0:4096], 0.0)
            for a in range(16):
                ch = a // 8
                nc.gpsimd.dma_scatter_add(
                    pbuf[2 * ch][:, :], xm[:, 2 * a:2 * a + 2, :],
                    idx_sb[:, a * 16:(a + 1) * 16],
                    256, 256, 64,
                    parity_reg=0, out_ap_other=pbuf[2 * ch + 1][:, :],
                    sbuf_tokens_per_rank=128,
                )

            # P_T [128, 160*64] bf16, slot t=16+y
            p_t = pool.tile([128, PT_SLOTS * 64], BF16, tag="p_t")
            nc.vector.memset(p_t[:, 0:PT_OFF * 64], 0.0)
            nc.vector.memset(p_t[:, (PT_OFF + 128) * 64:], 0.0)

            def pt_dst(t0):
                # [128, 64 slots step 2, 64] view into p_t starting at slot t0
                a = p_t[:, :]
                return AP(tensor=a.tensor, offset=a.offset + t0 * 64,
                          ap=[list(a.ap[0]), [128, 64], [1, 64]])

            nc.vector.tensor_tensor(
                pt_dst(PT_OFF),
                pbuf[0][:, 0:4096].rearrange("p (a b) -> p a b", b=64),
                pbuf[2][:, 0:4096].rearrange("p (a b) -> p a b", b=64), OP.add)
            nc.vector.tensor_tensor(
                pt_dst(PT_OFF + 1),
                pbuf[1][:, 0:4096].rearrange("p (a b) -> p a b", b=64),
                pbuf[3][:, 0:4096].rearrange("p (a b) -> p a b", b=64), OP.add)
            psum_stack.close()
            psum_stack = ExitStack()
            psa = psum_stack.enter_context(
                tc.tile_pool(name="psa", bufs=1, space="PSUM"))

            # ============== Phase A: AFDM dilation ==============
            cw_bf = pool.tile([128, 4, 128], BF16, tag="cw_bf")
            sw_bf = pool.tile([128, 4, 8], BF16, tag="sw_bf")
            nc.vector.tensor_copy(cw_bf[:, :, :], cw_view)
            nc.vector.tensor_copy(sw_bf[:, :, :], sw_view)

            # feature (nat2 layout) load + cast, chunked
            fnat_bf = pool.tile([128, 8192], BF16, tag="fnat_bf")
            for k in range(4):
                st = stpool.tile([128, 2048], F32, tag="fstage")
                nc.sync.dma_start(out=st[:, :],
                                  in_=feat_nat2[:, k * 2048:(k + 1) * 2048])
                nc.scalar.activation(fnat_bf[:, k * 2048:(k + 1) * 2048],
                                     st[:, :], AF.Copy)



            # conv1x1 (16 o-ch x 8 seg packed in M=128) + sf sums, 16 windows
            psum_r = psa.tile([128, 2048], F32, tag="psum_r")
            psum_sf = psa.tile([8, 2048], F32, tag="psum_sf")
            for wnd in range(16):
                g = wnd // 4
                st_flag = wnd < 4
                sp_flag = wnd >= 12
                rhs = fnat_bf[:, wnd * 512:(wnd + 1) * 512]
                nc.tensor.matmul(psum_r[:, (wnd % 4) * 512:(wnd % 4) * 512 + 512],
                                 cw_bf[:, g, :], rhs, start=st_flag, stop=sp_flag)
                nc.tensor.matmul(psum_sf[:, (wnd % 4) * 512:(wnd % 4) * 512 + 512],
                                 sw_bf[:, g, :], rhs, start=st_flag, stop=sp_flag)

            # r = psum_r + conv_b  (ACT Identity with bias col)
            r_sb = pool.tile([128, 2048], F32, tag="r_sb")
            nc.scalar.activation(r_sb[:, :], psum_r[:, :], AF.Identity,
                                 bias=cb_col[:, 0:1])
            # sf: min/max + pool
            sfmm = pool.tile([8, 2], F32, tag="sfmm")
            nc.vector.tensor_reduce(sfmm[:, 0:1], psum_sf[:, :], AX.X, OP.min)
            nc.vector.tensor_reduce(sfmm[:, 1:2], psum_sf[:, :], AX.X, OP.max)
            pool1sf = pool.tile([8, 256], F32, tag="pool1sf")
            nc.vector.tensor_reduce(pool1sf[:, :],
                                    psum_sf[:, :].rearrange("p (a b) -> p a b", b=8),
                                    AX.X, OP.add)
            pool2sf = pool.tile([8, 32], F32, tag="pool2sf")
            p1s = pool1sf[:, :]
            nc.vector.tensor_reduce(
                pool2sf[:, :].rearrange("p (a b) -> p a b", b=16),
                AP(tensor=p1s.tensor, offset=p1s.offset,
                   ap=[list(p1s.ap[0]), [128, 2], [1, 16], [16, 8]]),
                AX.X, OP.add)

            # close psa (psum_r / psum_sf fully consumed), open psb
            psum_stack.close()
            psum_stack = ExitStack()
            psb = psum_stack.enter_context(
                tc.tile_pool(name="psb", bufs=1, space="PSUM"))
            # stats: s1 = sum r, s2 = sum r^2 (ACT Square w/ accum)
            s_col = pool.tile([128, 2], F32, tag="s_col")
            nc.vector.tensor_reduce(s_col[:, 0:1], r_sb[:, :], AX.X, OP.add)
            rr_sb = pool.tile([128, 2048], BF16, tag="rr_sb")
            nc.scalar.activation(rr_sb[:, :], r_sb[:, :], AF.Square,
                                 accum_out=s_col[:, 1:2])
            psum_st = psb.tile([4, 2], F32, tag="psum_st")
            nc.tensor.matmul(psum_st[:, :], gm_v, s_col[:, :],
                             start=True, stop=True)
            # mu, rstd
            stt = pool.tile([4, 6], F32, tag="stt")
            inv_cnt = 1.0 / (4 * H * W)
            nc.vector.tensor_scalar_mul(stt[:, 0:1], psum_st[:, 0:1], inv_cnt)
            nc.vector.tensor_scalar_mul(stt[:, 1:2], psum_st[:, 1:2], inv_cnt)
            nc.vector.tensor_tensor(stt[:, 2:3], stt[:, 0:1], stt[:, 0:1], OP.mult)
            nc.vector.tensor_tensor(stt[:, 3:4], stt[:, 1:2], stt[:, 2:3], OP.subtract)
            nc.vector.tensor_scalar_add(stt[:, 3:4], stt[:, 3:4], EPS)
            nc.scalar.activation(stt[:, 4:5], stt[:, 3:4], AF.Sqrt)
            nc.vector.reciprocal(stt[:, 5:6], stt[:, 4:5])
            bc_in = pool.tile([4, 2], F32, tag="bc_in")
            nc.vector.tensor_copy(bc_in[:, 0:1], stt[:, 0:1])
            nc.vector.tensor_copy(bc_in[:, 1:2], stt[:, 5:6])
            psum_bc = psb.tile([128, 2], F32, tag="psum_bc")
            nc.tensor.matmul(psum_bc[:, :], gmt_v, bc_in[:, :],
                             start=True, stop=True)
            a_col = pool.tile([128, 1], F32, tag="a_col")
            b_col = pool.tile([128, 1], F32, tag="b_col")
            nc.vector.tensor_tensor(a_col[:, :], psum_bc[:, 1:2], gg_col[:, :], OP.mult)
            nc.vector.tensor_tensor(b_col[:, :], psum_bc[:, 0:1], a_col[:, :], OP.mult)
            nc.vector.tensor_tensor(b_col[:, :], gb_col[:, :], b_col[:, :], OP.subtract)
            # rr = relu(r*a + b)
            nc.scalar.activation(rr_sb[:, :], r_sb[:, :], AF.Relu,
                                 bias=b_col[:, 0:1], scale=a_col[:, 0:1])
            # pool 8x8
            pool1 = pool.tile([128, 256], F32, tag="pool1")
            nc.vector.tensor_reduce(pool1[:, :],
                                    rr_sb[:, :].rearrange("p (a b) -> p a b", b=8),
                                    AX.X, OP.add)
            pool2 = pool.tile([128, 32], F32, tag="pool2")
            p1a = pool1[:, :]
            nc.vector.tensor_reduce(
                pool2[:, :].rearrange("p (a b) -> p a b", b=16),
                AP(tensor=p1a.tensor, offset=p1a.offset,
                   ap=[list(p1a.ap[0]), [128, 2], [1, 16], [16, 8]]),
                AX.X, OP.add)

            mm2t = pool.tile([1, 16], F32, tag="mm2t")
            nc.sync.dma_start(out=mm2t[:, :], in_=sfmm[:, :])
            gmn = pool.tile([1, 4], F32, tag="gmn")
            mma = mm2t[:, :]
            nc.vector.tensor_reduce(
                gmn[:, 0:1],
                AP(tensor=mma.tensor, offset=mma.offset, ap=[list(mma.ap[0]), [2, 8]]),
                AX.X, OP.min)
            nc.vector.tensor_reduce(
                gmn[:, 1:2],
                AP(tensor=mma.tensor, offset=mma.offset + 1,
                   ap=[list(mma.ap[0]), [2, 8]]),
                AX.X, OP.max)
            nc.vector.tensor_tensor(gmn[:, 2:3], gmn[:, 1:2], gmn[:, 0:1], OP.subtract)
            nc.vector.tensor_scalar_add(gmn[:, 2:3], gmn[:, 2:3], EPS)
            nc.vector.reciprocal(gmn[:, 3:4], gmn[:, 2:3])
            pack12 = pool.tile([1, 2], F32, tag="pack12")
            nc.vector.tensor_copy(pack12[:, 0:1], gmn[:, 0:1])
            nc.vector.tensor_copy(pack12[:, 1:2], gmn[:, 3:4])
            psum_sc = psb.tile([128, 2], F32, tag="psum_sc")
            nc.tensor.matmul(psum_sc[:, :], o1128_v, pack12[:, :],
                             start=True, stop=True)
            sc_sb = pool.tile([128, 2], F32, tag="sc_sb")
            nc.vector.tensor_copy(sc_sb[:, :], psum_sc[:, :])
            psum_sfbc = psb.tile([128, 32], F32, tag="psum_sfbc")
            nc.tensor.matmul(psum_sfbc[:, :], gr8_v, pool2sf[:, :],
                             start=True, stop=True)
            sfterm = pool.tile([128, 32], F32, tag="sfterm")
            nc.vector.tensor_scalar(sfterm[:, :], psum_sfbc[:, :],
                                    1.0 / 64, sc_sb[:, 0:1], OP.mult, OP.subtract)
            nc.vector.tensor_scalar_mul(sfterm[:, :], sfterm[:, :], sc_sb[:, 1:2])
            flat_f = pool.tile([128, 32], F32, tag="flat_f")
            nc.vector.tensor_scalar_mul(flat_f[:, :], pool2[:, :], 1.0 / 64)
            nc.vector.tensor_tensor(flat_f[:, :], flat_f[:, :], sfterm[:, :], OP.add)
            flat_bf = pool.tile([128, 32], BF16, tag="flat_bf")
            nc.vector.tensor_copy(flat_bf[:, :], flat_f[:, :])

            # w1n load + cast, chunked
            w1_bf = pool.tile([128, 32, 128], BF16, tag="w1_bf")
            for k in range(4):
                st = stpool.tile([128, 8, 128], F32, tag="wstage")
                nc.sync.dma_start(out=st[:, :, :],
                                  in_=w1n[:, 8 * k:8 * k + 8, :])
                nc.scalar.activation(w1_bf[:, 8 * k:8 * k + 8, :],
                                     st[:, :, :], AF.Copy)
            # MLP
            psum_h = psb.tile([1, 128], F32, tag="psum_h")
            for j in range(32):
                nc.tensor.matmul(psum_h[:, :], flat_bf[:, j:j + 1],
                                 w1_bf[:, j, :], start=(j == 0), stop=(j == 31))
            hr = pool.tile([1, 128], F32, tag="hr")
            nc.scalar.activation(hr[:, :], psum_h[:, :], AF.Relu)
            hw2 = pool.tile([1, 128], F32, tag="hw2")
            nc.vector.tensor_tensor(hw2[:, :], hr[:, :], w2_v, OP.mult)
            dsc = pool.tile([1, 4], F32, tag="dsc")
            nc.vector.tensor_reduce(dsc[:, 0:1], hw2[:, :], AX.X, OP.add)
            nc.scalar.activation(dsc[:, 1:2], dsc[:, 0:1], AF.Sigmoid)
            nc.vector.tensor_scalar_mul(dsc[:, 2:3], dsc[:, 1:2], float(MD))
            nc.vector.reciprocal(dsc[:, 3:4], dsc[:, 2:3])

            # w' = exp(-dist/d)/sum (+1 center)
            psum_i11 = psb.tile([11, 1], F32, tag="psum_i11")
            nc.tensor.matmul(psum_i11[:, :], o111_v, dsc[:, 3:4],
                             start=True, stop=True)
            invd_col = pool.tile([11, 1], F32, tag="invd_col")
            nc.vector.tensor_copy(invd_col[:, :], psum_i11[:, :])
            wexp = pool.tile([11, 11], F32, tag="wexp")
            nc.scalar.activation(wexp[:, :], dnt_v, AF.Exp,
                                 scale=invd_col[:, 0:1])
            psum_ws = psb.tile([1, 11], F32, tag="psum_ws")
            nc.tensor.matmul(psum_ws[:, :], o11_v, wexp[:, :],
                             start=True, stop=True)
            wsv = pool.tile([1, 2], F32, tag="wsv")
            nc.vector.tensor_reduce(wsv[:, 0:1], psum_ws[:, :], AX.X, OP.add)
            nc.vector.reciprocal(wsv[:, 1:2], wsv[:, 0:1])
            psum_w11 = psb.tile([11, 1], F32, tag="psum_w11")
            nc.tensor.matmul(psum_w11[:, :], o111_v, wsv[:, 1:2],
                             start=True, stop=True)
            wsi_col = pool.tile([11, 1], F32, tag="wsi_col")
            nc.vector.tensor_copy(wsi_col[:, :], psum_w11[:, :])
            wp_sb = pool.tile([11, 400], F32, tag="wp_sb")
            nc.vector.memset(wp_sb[:, :], 0.0)
            nc.vector.tensor_scalar_mul(wp_sb[:, 250:261], wexp[:, :],
                                        wsi_col[:, 0:1])
            nc.vector.tensor_tensor(wp_sb[:, 250:261], wp_sb[:, 250:261],
                                    cm_v, OP.add)
            nc.sync.dma_start(out=vdram[:, :], in_=wp_sb[:, :])

            # T matrices via sliding-window DMA + fold
            t_ext = pool.tile([128, 11, 138], F32, tag="t_ext")
            # Load T with all-positive steps (contiguous 552B runs) by
            # storing j REVERSED: t_ext[p, dy, jr] = v[dy, 123 + p + jr]
            # (valid because the kernel rows are symmetric in dx, so the
            # reversed generator equals the original). xe = 132 - jr.
            nc.gpsimd.dma_start(
                out=t_ext[:, :, :],
                in_=AP(tensor=vdram, offset=123,
                       ap=[[1, 128], [400, 11], [1, 138]]))
            # folds in reversed coords: xe=0 target at jr=132 (sources
            # jr 133..137), xe=127 target at jr=5 (sources jr 0..4)
            tl = pool.tile([128, 11, 1], F32, tag="tl")
            th = pool.tile([128, 11, 1], F32, tag="th")
            nc.vector.tensor_reduce(tl[:, :, :], t_ext[:, :, 133:138], AX.X, OP.add)
            nc.vector.tensor_reduce(th[:, :, :], t_ext[:, :, 0:5], AX.X, OP.add)
            nc.vector.tensor_tensor(t_ext[:, :, 132:133], t_ext[:, :, 132:133],
                                    tl[:, :, :], OP.add)
            nc.vector.tensor_tensor(t_ext[:, :, 5:6], t_ext[:, :, 5:6],
                                    th[:, :, :], OP.add)
            t_fold = pool.tile([128, 11, 128], BF16, tag="t_fold")
            tea = t_ext[:, :, :]
            nc.vector.tensor_copy(
                t_fold[:, :, :],
                AP(tensor=tea.tensor, offset=tea.offset + 132,
                   ap=[list(tea.ap[0]), [138, 11], [-1, 128]]))

            # ============== Phase C: Toeplitz conv + tail ==============
            psum_stack.close()
            psum_stack = ExitStack()
            psc = psum_stack.enter_context(
                tc.tile_pool(name="psc", bufs=2, space="PSUM"))
            ft = pool.tile([128, 8192], F32, tag="ft")

            p_t_flat = p_t[:, :]
            for ps in range(5):
                nslots = 32 if ps < 4 else 16
                psum_c = psc.tile([128, nslots * 64], F32, tag="psum_c")
                for dyi in range(11):
                    dy = dyi - 5
                    for ch in range(nslots // 8):
                        u0 = 32 * ps + 8 * ch
                        off = (u0 + 8 - dy) * 64
                        rhs = p_t_flat[:, off:off + 512].rearrange("p a -> p a")
                        nc.tensor.matmul(
                            psum_c[:, ch * 512:ch * 512 + 512],
                            t_fold[:, dyi, :], rhs,
                            start=(dyi == 0), stop=(dyi == 10))
                # tail: feat_T += psum slice  (real y rows of this pass)
                y0 = max(0, 32 * ps - 8)
                y1 = min(128, 32 * ps + 24)
                nc.sync.dma_start(out=ft[:, y0 * 64:y1 * 64],
                                  in_=feat_T[:, y0 * 64:y1 * 64])
                po = (y0 + 8 - 32 * ps) * 64
                nc.vector.tensor_tensor(
                    ft[:, y0 * 64:y1 * 64],
                    psum_c[:, po:po + (y1 - y0) * 64],
                    ft[:, y0 * 64:y1 * 64], OP.add)
                if ps == 0:
                    pa = psum_c[:, :]
                    tmpf = pool.tile([128, 64], F32, tag="tmpf")
                    nc.vector.tensor_reduce(
                        tmpf[:, :],
                        AP(tensor=pa.tensor, offset=pa.offset + 3 * 64,
                           ap=[list(pa.ap[0]), [1, 64], [64, 5]]),
                        AX.X, OP.add)
                    nc.vector.tensor_tensor(ft[:, 0:64], ft[:, 0:64],
                                            tmpf[:, :], OP.add)
                if ps == 4:
                    pa = psum_c[:, :]
                    tmph = pool.tile([128, 64], F32, tag="tmph")
                    nc.vector.tensor_reduce(
                        tmph[:, :],
                        AP(tensor=pa.tensor, offset=pa.offset + 8 * 64,
                           ap=[list(pa.ap[0]), [1, 64], [64, 5]]),
                        AX.X, OP.add)
                    nc.vector.tensor_tensor(ft[:, 127 * 64:128 * 64],
                                            ft[:, 127 * 64:128 * 64],
                                            tmph[:, :], OP.add)
                nc.sync.dma_start(out=out_T[:, y0 * 64:y1 * 64],
                                  in_=ft[:, y0 * 64:y1 * 64])

            psum_stack.close()

    nc.compile()
    return nc


def build_core_inputs(x, xyz, feature, conv_w, conv_b, gn_gamma, gn_beta,
                      mlp_w1, mlp_w2):
    """Host-side sharding glue: slice batch b per core + layout transforms."""
    f32 = np.float32
    # shared constants
    convw_pl = np.zeros((128, 4, 128), f32)
    sfw_pl = np.zeros((128, 4, 8), f32)
    for h in range(2):
        for g in range(4):
            convw_pl[64 * h:64 * h + 64, g, np.arange(16) * 8 + h * 4 + g] = conv_w.T
            sfw_pl[64 * h:64 * h + 64, g, h * 4 + g] = 1.0
    gmat = (np.arange(128)[:, None] // 32 == np.arange(4)[None, :]).astype(f32)
    ltri_m = (np.arange(128)[:, None] < np.arange(128)[None, :]).astype(f32)
    trash_m = (16384 + (np.arange(32)[None, :] % 2) * 128
               + np.arange(128)[:, None]).astype(f32)
    ident_m = np.eye(128, dtype=f32)
    selw_m = np.zeros((128, 128), f32)
    for g in range(8):
        for q in range(16):
            selw_m[16 * g + q, g * 16 + q] = 1.0
    blob1 = np.zeros((128, 968), f32)
    blob1[:, 0:128] = ltri_m
    blob1[:, 128:256] = ident_m
    blob1[:, 256:384] = selw_m
    blob1[:, 384:896] = convw_pl.reshape(128, 512)
    blob1[:, 896:928] = trash_m
    blob1[:, 928:960] = sfw_pl.reshape(128, 32)
    blob1[:, 960:964] = gmat
    blob1[:, 964] = np.repeat(conv_b, 8)
    blob1[:, 965] = np.repeat(gn_gamma, 8)
    blob1[:, 966] = np.repeat(gn_beta, 8)
    blob1[:, 967] = 1.0
    dxy = np.arange(11) - 5
    blob2 = np.zeros((11, 546), f32)
    blob2[0:4, 0:128] = gmat.T
    blob2[0:8, 128:256] = (np.arange(128)[None, :] % 8
                           == np.arange(8)[:, None]).astype(f32)
    blob2[0, 256:384] = mlp_w2[0]
    blob2[0, 384:512] = 1.0
    blob2[0:11, 512:523] = -np.sqrt(dxy[None, :] ** 2 + dxy[:, None] ** 2)
    blob2[5, 523 + 5] = 1.0
    blob2[0:11, 534] = 1.0
    blob2[0, 535:546] = 1.0
    # w1 rearranged: w1n[p=(o,seg), j=(by_l,bx), n] = mlp_w1[n, o*256+(seg*2+by_l)*16+bx]
    o = np.arange(16)[:, None, None, None]
    seg = np.arange(8)[None, :, None, None]
    byl = np.arange(2)[None, None, :, None]
    bx = np.arange(16)[None, None, None, :]
    fl = (o * 256 + (seg * 2 + byl) * 16 + bx).reshape(128, 32)
    w1n = np.ascontiguousarray(mlp_w1.T[fl]).astype(f32)  # [128, 32, 128]
    shared = dict(blob1=blob1, blob2=blob2, w1n=w1n)

    in_maps = []
    for b in range(B):
        fb = np.ascontiguousarray(feature[b].reshape(64, 16384)).astype(f32)
        m = dict(shared)
        m["x_wrap"] = np.ascontiguousarray(
            x[b].reshape(32, 128, 64).transpose(1, 0, 2)).astype(f32)
        m["xx_w2"] = np.ascontiguousarray(xyz[b, :, 0].reshape(32, 128).T).astype(f32)
        m["xy_w2"] = np.ascontiguousarray(xyz[b, :, 1].reshape(32, 128).T).astype(f32)
        m["feat_nat2"] = np.concatenate([fb[:, :8192], fb[:, 8192:]], axis=0)
        m["feat_T"] = np.ascontiguousarray(
            feature[b].transpose(2, 1, 0).reshape(128, 8192)).astype(f32)
        in_maps.append(m)
    return in_maps


_NC_CACHE = {}


def kernel(x, xyz, feature, conv_w, conv_b, gn_gamma, gn_beta, mlp_w1, mlp_w2,
           _trace=False):
    from concourse.bass_utils import run_bass_kernel_spmd
    if "nc" not in _NC_CACHE:
        _NC_CACHE["nc"] = build_nc()
    nc = _NC_CACHE["nc"]
    in_maps = build_core_inputs(np.asarray(x), np.asarray(xyz),
                                np.asarray(feature), np.asarray(conv_w),
                                np.asarray(conv_b), np.asarray(gn_gamma),
                                np.asarray(gn_beta), np.asarray(mlp_w1),
                                np.asarray(mlp_w2))
    res = run_bass_kernel_spmd(nc, in_maps, core_ids=list(range(8)),
                               trace=_trace)
    outs = []
    for i in range(B):
        ot = res.results[i]["out_T"]
        outs.append(ot.reshape(128, 128, 64).transpose(2, 1, 0))
    out = np.stack(outs).astype(np.float32)
    if _trace:
        return out, res
    return out



# revision 3
# speedup vs baseline: 1.0069x; 1.0069x over previous
"""Trainium2 Bass kernel for nn_AMM_w_AFDM (scatter_memory).

Strategy (one batch per NeuronCore, 8 cores data-parallel):
  out[b] = feature + P + splat(P, w)  where P = nearest-cell scatter of x.
  The 11x11 splat-with-border-clipping is computed as a bank of banded
  Toeplitz matmuls on TensorE; P is built with gpsimd.dma_scatter_add
  (SBUF parity-split destination), pipelined per-half with the dedup so
  the gpsimd descriptor generation overlaps the dedup math. The AFDM
  dilation scalar is computed on-device and overlaps the scatter window.
  Inputs feature/x/w1 are pre-cast to bf16 host-side; output is bf16.
"""
import sys
from contextlib import ExitStack
import numpy as np

sys.path.insert(0, "/opt/trn_rl_repo")

import concourse.bacc as bacc  # noqa: E402
import concourse.bass as bass  # noqa: E402
import concourse.mybir as mybir  # noqa: E402
import concourse.tile as tile  # noqa: E402
from concourse.ap import AP  # noqa: E402

MD = 5
EPS = 1e-5
B, N, C, H, W = 8, 4096, 64, 128, 128
F32 = mybir.dt.float32
BF16 = mybir.dt.bfloat16
I16 = mybir.dt.int16
AX = mybir.AxisListType
OP = mybir.AluOpType
AF = mybir.ActivationFunctionType

# P_T slot layout: slot t in [0,160), y = t-16 (zeros outside [0,128)).
PT_SLOTS = 160
PT_OFF = 16
# ext output slots: u in [0,144), ye = u-8.
EXT_SLOTS = 144


def build_nc():
    nc = bacc.Bacc("TRN2", target_bir_lowering=False)

    def din(name, shape, dt=F32):
        return nc.dram_tensor(name, shape, dt, kind="ExternalInput")

    x_wrap = din("x_wrap", [128, 32, 64], BF16)
    xx_w2 = din("xx_w2", [128, 32])
    xy_w2 = din("xy_w2", [128, 32])
    feat_nat2 = din("feat_nat2", [128, 8192], BF16)
    feat_T = din("feat_T", [128, 8192], BF16)
    blob1 = din("blob1", [128, 968])
    blob2 = din("blob2", [11, 546])
    w1n = din("w1n", [128, 32, 128], BF16)
    lint_dram = nc.dram_tensor("lint_dram", [32, 128], F32)

    out_T = nc.dram_tensor("out_T", [128, 8192], BF16, kind="ExternalOutput")
    vdram = nc.dram_tensor("vdram", [11, 400], F32)

    with tile.TileContext(nc) as tc:
        with tc.tile_pool(name="main", bufs=1) as pool, \
             tc.tile_pool(name="scat", bufs=1) as scpool, \
             tc.tile_pool(name="stage", bufs=2) as stpool:
            psum_stack = ExitStack()

            # ============== early input DMAs ==============
            ps_prep = psum_stack.enter_context(
                tc.tile_pool(name="psp", bufs=2, space="PSUM"))
            xx2 = pool.tile([128, 32], F32, tag="xx2")
            xy2 = pool.tile([128, 32], F32, tag="xy2")
            nc.sync.dma_start(out=xx2[:, :], in_=xx_w2[:, :])
            nc.sync.dma_start(out=xy2[:, :], in_=xy_w2[:, :])
            blob1_sb = pool.tile([128, 968], F32, tag="blob1_sb")
            nc.sync.dma_start(out=blob1_sb[:, :], in_=blob1[:, :])
            blob2_sb = pool.tile([11, 546], F32, tag="blob2_sb")
            nc.sync.dma_start(out=blob2_sb[:, :], in_=blob2[:, :])
            x_bf = scpool.tile([128, 32, 64], BF16, tag="x_bf")
            nc.sync.dma_start(out=x_bf[:, :, :], in_=x_wrap[:, :, :])
            fnat_bf = pool.tile([128, 8192], BF16, tag="fnat_bf")
            nc.sync.dma_start(out=fnat_bf[:, :], in_=feat_nat2[:, :])
            w1_bf = pool.tile([128, 32, 128], BF16, tag="w1_bf")
            nc.sync.dma_start(out=w1_bf[:, :, :], in_=w1n[:, :, :])

            # scatter destination buffers: memset on DVE, early
            pb0 = scpool.tile([128, 66 * 64], BF16, tag="pb0")
            pb1 = scpool.tile([128, 66 * 64], BF16, tag="pb1")
            pb2 = scpool.tile([128, 66 * 64], BF16, tag="pb2")
            pb3 = scpool.tile([128, 66 * 64], BF16, tag="pb3")
            pbuf = [pb0, pb1, pb2, pb3]  # [A_even, A_odd, B_even, B_odd]
            for pb in pbuf:
                nc.vector.memset(pb[:, 0:4096], 0.0)
            # P_T border slots zeroed early too
            p_t = pool.tile([128, PT_SLOTS * 64], BF16, tag="p_t")
            nc.vector.memset(p_t[:, 0:PT_OFF * 64], 0.0)
            nc.vector.memset(p_t[:, (PT_OFF + 128) * 64:], 0.0)

            # ============== Phase S: scatter x -> P ==============
            # Index math in the 128-wrap: lin128[p, t] for point j = t*128+p.
            def floor127(srcw, sfx):
                # floor(t) = round(t) - (round(t) > t), round via +/- 2^23
                t = pool.tile([128, 32], F32, tag=f"fl_t{sfx}")
                r = pool.tile([128, 32], F32, tag=f"fl_r{sfx}")
                g = pool.tile([128, 32], F32, tag=f"fl_g{sfx}")
                o = pool.tile([128, 32], F32, tag=f"fl_o{sfx}")
                nc.vector.tensor_scalar_mul(t[:, :], srcw[:, :], 127.0)
                nc.vector.tensor_scalar(r[:, :], t[:, :], 8388608.0, -8388608.0,
                                        OP.add, OP.add)
                nc.vector.tensor_tensor(g[:, :], r[:, :], t[:, :], OP.is_gt)
                nc.vector.tensor_tensor(o[:, :], r[:, :], g[:, :], OP.subtract)
                return o

            xc = floor127(xx2, "x")
            yc = floor127(xy2, "y")
            lin128 = pool.tile([128, 32], F32, tag="lin128")
            nc.vector.tensor_scalar_mul(lin128[:, :], yc[:, :], 128.0)
            nc.vector.tensor_tensor(lin128[:, :], lin128[:, :], xc[:, :], OP.add)

            ltri_f = blob1_sb[:, 0:128]
            ident_v = blob1_sb[:, 128:256]
            selw_v = blob1_sb[:, 256:384]
            cw_view = blob1_sb[:, 384:896].rearrange("p (a b) -> p a b", b=128)
            trash_sb = blob1_sb[:, 896:928]
            sw_view = blob1_sb[:, 928:960].rearrange("p (a b) -> p a b", b=8)
            gm_v = blob1_sb[:, 960:964]
            cb_col = blob1_sb[:, 964:965]
            gg_col = blob1_sb[:, 965:966]
            gb_col = blob1_sb[:, 966:967]
            onesc_f = blob1_sb[:, 967:968]
            gmt_v = blob2_sb[0:4, 0:128]
            gr8_v = blob2_sb[0:8, 128:256]
            w2_v = blob2_sb[0:1, 256:384]
            o1128_v = blob2_sb[0:1, 384:512]
            dnt_v = blob2_sb[0:11, 512:523]
            cm_v = blob2_sb[0:11, 523:534]
            o11_v = blob2_sb[0:11, 534:535]
            o111_v = blob2_sb[0:1, 535:546]
            ltri_sb = pool.tile([128, 128], BF16, tag="ltri_sb")
            nc.vector.tensor_copy(ltri_sb[:, :], ltri_f)
            onesc_bf = pool.tile([128, 1], BF16, tag="onesc_bf")
            nc.vector.tensor_copy(onesc_bf[:, :], onesc_f)

            xm = scpool.tile([128, 32, 64], BF16, tag="xm")

            # Per-256-chunk dedup: chunk a = slots (2a, 2a+1) = subs (u, v).
            first128 = pool.tile([128, 32], F32, tag="first128")
            ps_lt = ps_prep.tile([32, 128], F32, tag="ps_lt")
            nc.tensor.transpose(ps_lt[:, :], lin128[:, :], ident_v)
            linT = pool.tile([32, 128], F32, tag="linT")
            nc.vector.tensor_copy(linT[:, :], ps_lt[:, :])
            nc.sync.dma_start(out=lint_dram[:, :], in_=linT[:, :])

            def eq_mat(out_bf, bc_psum, bc_off, col_t):
                # out[q, p] = (lin(col_t, q) == bcast[bc_off + p])
                nc.vector.tensor_scalar(out_bf[:, :], bc_psum[:, bc_off:bc_off + 128],
                                        lin128[:, col_t:col_t + 1], None, OP.is_equal)

            idxf = pool.tile([128, 32], F32, tag="idxf")
            idx_sb = pool.tile([128, 256], I16, tag="idx")
            idxw_f = pool.tile([16, 256], F32, tag="idxw_f")

            for hf in range(2):
                for a4 in range(4 * hf, 4 * hf + 4):  # 4 slots per bcast matmul
                    lrow = stpool.tile([1, 512], F32, tag="lrow")
                    nc.sync.dma_start(
                        out=lrow[:, :],
                        in_=AP(tensor=lint_dram, offset=a4 * 512,
                               ap=[[1, 1], [1, 512]]))
                    bc = ps_prep.tile([128, 512], F32, tag="bc")
                    nc.tensor.matmul(bc[:, :], o1128_v, lrow[:, :],
                                     start=True, stop=True)
                    for ci in range(2):
                        a = a4 * 2 + ci
                        u, v = 2 * a, 2 * a + 1
                        uoff, voff = (u % 4) * 128, (v % 4) * 128
                        m_uu = pool.tile([128, 128], BF16, tag="m_uu")
                        m_vv = pool.tile([128, 128], BF16, tag="m_vv")
                        m_uv = pool.tile([128, 128], BF16, tag="m_uv")
                        m_vu = pool.tile([128, 128], BF16, tag="m_vu")
                        eq_mat(m_uu, bc, uoff, u)   # rows q: sub u, cols p: sub u
                        eq_mat(m_vv, bc, voff, v)   # rows: v, cols: v
                        eq_mat(m_uv, bc, voff, u)   # rows: u, cols: v
                        eq_mat(m_vu, bc, uoff, v)   # rows: v, cols: u
                        # merged_u = m_uu.T @ x_u + m_vu.T @ x_v
                        pm = ps_prep.tile([128, 128], F32, tag="pm")
                        nc.tensor.matmul(pm[:, 0:64], m_uu[:, :], x_bf[:, u, :],
                                         start=True, stop=False)
                        nc.tensor.matmul(pm[:, 0:64], m_vu[:, :], x_bf[:, v, :],
                                         start=False, stop=True)
                        nc.tensor.matmul(pm[:, 64:128], m_vv[:, :], x_bf[:, v, :],
                                         start=True, stop=True)
                        nc.scalar.activation(xm[:, u:u + 2, :], pm[:, :], AF.Copy)
                        # counts: below_u = (m_uu*L).T @ 1 ; below_v = (m_vv*L).T @ 1
                        #         + m_uv.T @ 1 (any match in sub u)
                        ml_u = pool.tile([128, 128], BF16, tag="ml_u")
                        ml_v = pool.tile([128, 128], BF16, tag="ml_v")
                        nc.vector.tensor_tensor(ml_u[:, :], m_uu[:, :], ltri_sb[:, :],
                                                OP.mult)
                        nc.vector.tensor_tensor(ml_v[:, :], m_vv[:, :], ltri_sb[:, :],
                                                OP.mult)
                        cnt = ps_prep.tile([128, 2], F32, tag="cnt")
                        nc.tensor.matmul(cnt[:, 0:1], ml_u[:, :], onesc_bf[:, :],
                                         start=True, stop=True)
                        nc.tensor.matmul(cnt[:, 1:2], ml_v[:, :], onesc_bf[:, :],
                                         start=True, stop=False)
                        nc.tensor.matmul(cnt[:, 1:2], m_uv[:, :], onesc_bf[:, :],
                                         start=False, stop=True)
                        nc.vector.tensor_scalar(first128[:, u:u + 2], cnt[:, :],
                                                0.5, None, OP.is_lt)

                # idx = first ? lin : trash for this half, then scatter chain
                hs = slice(16 * hf, 16 * hf + 16)
                nc.vector.tensor_tensor(idxf[:, hs], lin128[:, hs],
                                        trash_sb[:, hs], OP.subtract)
                nc.vector.tensor_tensor(idxf[:, hs], idxf[:, hs],
                                        first128[:, hs], OP.mult)
                nc.vector.tensor_tensor(idxf[:, hs], idxf[:, hs],
                                        trash_sb[:, hs], OP.add)
                # 16-wrap: idxw[q, t*8+g] = idxf[16g+q, t]
                for g in range(8):
                    ps_w = ps_prep.tile([16, 16], F32, tag="bc")
                    nc.tensor.matmul(ps_w[:, :], selw_v[:, 16 * g:16 * g + 16],
                                     idxf[:, hs], start=True, stop=True)
                    dst = idxw_f[:, :]
                    nc.vector.tensor_copy(
                        AP(tensor=dst.tensor, offset=dst.offset + 128 * hf + g,
                           ap=[list(dst.ap[0]), [8, 16]]),
                        ps_w[:, :])
                nc.vector.tensor_copy(idx_sb[0:16, 128 * hf:128 * hf + 128],
                                      idxw_f[:, 128 * hf:128 * hf + 128])
                for r in range(1, 8):
                    nc.sync.dma_start(
                        out=idx_sb[16 * r:16 * r + 16, 128 * hf:128 * hf + 128],
                        in_=idx_sb[0:16, 128 * hf:128 * hf + 128])

                # scatter this half's 8 chunks; alternate chains for DMA overlap
                for a in range(8 * hf, 8 * hf + 8):
                    ch = a % 2
                    nc.gpsimd.dma_scatter_add(
                        pbuf[2 * ch][:, :], xm[:, 2 * a:2 * a + 2, :],
                        idx_sb[:, a * 16:(a + 1) * 16],
                        256, 256, 64,
                        parity_reg=0, out_ap_other=pbuf[2 * ch + 1][:, :],
                        sbuf_tokens_per_rank=128,
                    )

            psum_stack.close()
            psum_stack = ExitStack()
            psa = psum_stack.enter_context(
                tc.tile_pool(name="psa", bufs=1, space="PSUM"))

            # ============== Phase A: AFDM dilation ==============
            # (emitted before the P merge so its DVE/ACT/PE ops run during the
            #  scatter window)
            cw_bf = pool.tile([128, 4, 128], BF16, tag="cw_bf")
            sw_bf = pool.tile([128, 4, 8], BF16, tag="sw_bf")
            nc.vector.tensor_copy(cw_bf[:, :, :], cw_view)
            nc.vector.tensor_copy(sw_bf[:, :, :], sw_view)

            # conv1x1 (16 o-ch x 8 seg packed in M=128) + sf sums, 16 windows
            psum_r = psa.tile([128, 2048], F32, tag="psum_r")
            psum_sf = psa.tile([8, 2048], F32, tag="psum_sf")
            for wnd in range(16):
                g = wnd // 4
                st_flag = wnd < 4
                sp_flag = wnd >= 12
                rhs = fnat_bf[:, wnd * 512:(wnd + 1) * 512]
                nc.tensor.matmul(psum_r[:, (wnd % 4) * 512:(wnd % 4) * 512 + 512],
                                 cw_bf[:, g, :], rhs, start=st_flag, stop=sp_flag)
                nc.tensor.matmul(psum_sf[:, (wnd % 4) * 512:(wnd % 4) * 512 + 512],
                                 sw_bf[:, g, :], rhs, start=st_flag, stop=sp_flag)

            # r = psum_r + conv_b  (ACT Identity with bias col)
            r_sb = pool.tile([128, 2048], F32, tag="r_sb")
            nc.scalar.activation(r_sb[:, :], psum_r[:, :], AF.Identity,
                                 bias=cb_col[:, 0:1])
            # sf: min/max + pool
            sfmm = pool.tile([8, 2], F32, tag="sfmm")
            nc.vector.tensor_reduce(sfmm[:, 0:1], psum_sf[:, :], AX.X, OP.min)
            nc.vector.tensor_reduce(sfmm[:, 1:2], psum_sf[:, :], AX.X, OP.max)
            pool1sf = pool.tile([8, 256], F32, tag="pool1sf")
            nc.vector.tensor_reduce(pool1sf[:, :],
                                    psum_sf[:, :].rearrange("p (a b) -> p a b", b=8),
                                    AX.X, OP.add)
            pool2sf = pool.tile([8, 32], F32, tag="pool2sf")
            p1s = pool1sf[:, :]
            nc.vector.tensor_reduce(
                pool2sf[:, :].rearrange("p (a b) -> p a b", b=16),
                AP(tensor=p1s.tensor, offset=p1s.offset,
                   ap=[list(p1s.ap[0]), [128, 2], [1, 16], [16, 8]]),
                AX.X, OP.add)

            # close psa (psum_r / psum_sf fully consumed), open psb
            psum_stack.close()
            psum_stack = ExitStack()
            psb = psum_stack.enter_context(
                tc.tile_pool(name="psb", bufs=1, space="PSUM"))
            # stats: s1 = sum r, s2 = sum r^2 (ACT Square w/ accum)
            s_col = pool.tile([128, 2], F32, tag="s_col")
            nc.vector.tensor_reduce(s_col[:, 0:1], r_sb[:, :], AX.X, OP.add)
            rr_sb = pool.tile([128, 2048], BF16, tag="rr_sb")
            nc.scalar.activation(rr_sb[:, :], r_sb[:, :], AF.Square,
                                 accum_out=s_col[:, 1:2])
            psum_st = psb.tile([4, 2], F32, tag="psum_st")
            nc.tensor.matmul(psum_st[:, :], gm_v, s_col[:, :],
                             start=True, stop=True)
            # mu, rstd
            stt = pool.tile([4, 6], F32, tag="stt")
            inv_cnt = 1.0 / (4 * H * W)
            nc.vector.tensor_scalar_mul(stt[:, 0:1], psum_st[:, 0:1], inv_cnt)
            nc.vector.tensor_scalar_mul(stt[:, 1:2], psum_st[:, 1:2], inv_cnt)
            nc.vector.tensor_tensor(stt[:, 2:3], stt[:, 0:1], stt[:, 0:1], OP.mult)
            nc.vector.tensor_tensor(stt[:, 3:4], stt[:, 1:2], stt[:, 2:3], OP.subtract)
            nc.vector.tensor_scalar_add(stt[:, 3:4], stt[:, 3:4], EPS)
            nc.scalar.activation(stt[:, 4:5], stt[:, 3:4], AF.Sqrt)
            nc.vector.reciprocal(stt[:, 5:6], stt[:, 4:5])
            bc_in = pool.tile([4, 2], F32, tag="bc_in")
            nc.vector.tensor_copy(bc_in[:, 0:1], stt[:, 0:1])
            nc.vector.tensor_copy(bc_in[:, 1:2], stt[:, 5:6])
            psum_bc = psb.tile([128, 2], F32, tag="psum_bc")
            nc.tensor.matmul(psum_bc[:, :], gmt_v, bc_in[:, :],
                             start=True, stop=True)
            a_col = pool.tile([128, 1], F32, tag="a_col")
            b_col = pool.tile([128, 1], F32, tag="b_col")
            nc.vector.tensor_tensor(a_col[:, :], psum_bc[:, 1:2], gg_col[:, :], OP.mult)
            nc.vector.tensor_tensor(b_col[:, :], psum_bc[:, 0:1], a_col[:, :], OP.mult)
            nc.vector.tensor_tensor(b_col[:, :], gb_col[:, :], b_col[:, :], OP.subtract)
            # rr = relu(r*a + b)
            nc.scalar.activation(rr_sb[:, :], r_sb[:, :], AF.Relu,
                                 bias=b_col[:, 0:1], scale=a_col[:, 0:1])
            # pool 8x8
            pool1 = pool.tile([128, 256], F32, tag="pool1")
            nc.vector.tensor_reduce(pool1[:, :],
                                    rr_sb[:, :].rearrange("p (a b) -> p a b", b=8),
                                    AX.X, OP.add)
            pool2 = pool.tile([128, 32], F32, tag="pool2")
            p1a = pool1[:, :]
            nc.vector.tensor_reduce(
                pool2[:, :].rearrange("p (a b) -> p a b", b=16),
                AP(tensor=p1a.tensor, offset=p1a.offset,
                   ap=[list(p1a.ap[0]), [128, 2], [1, 16], [16, 8]]),
                AX.X, OP.add)

            mm2t = pool.tile([1, 16], F32, tag="mm2t")
            nc.sync.dma_start(out=mm2t[:, :], in_=sfmm[:, :])
            gmn = pool.tile([1, 4], F32, tag="gmn")
            mma = mm2t[:, :]
            nc.vector.tensor_reduce(
                gmn[:, 0:1],
                AP(tensor=mma.tensor, offset=mma.offset, ap=[list(mma.ap[0]), [2, 8]]),
                AX.X, OP.min)
            nc.vector.tensor_reduce(
                gmn[:, 1:2],
                AP(tensor=mma.tensor, offset=mma.offset + 1,
                   ap=[list(mma.ap[0]), [2, 8]]),
                AX.X, OP.max)
            nc.vector.tensor_tensor(gmn[:, 2:3], gmn[:, 1:2], gmn[:, 0:1], OP.subtract)
            nc.vector.tensor_scalar_add(gmn[:, 2:3], gmn[:, 2:3], EPS)
            nc.vector.reciprocal(gmn[:, 3:4], gmn[:, 2:3])
            pack12 = pool.tile([1, 2], F32, tag="pack12")
            nc.vector.tensor_copy(pack12[:, 0:1], gmn[:, 0:1])
            nc.vector.tensor_copy(pack12[:, 1:2], gmn[:, 3:4])
            psum_sc = psb.tile([128, 2], F32, tag="psum_sc")
            nc.tensor.matmul(psum_sc[:, :], o1128_v, pack12[:, :],
                             start=True, stop=True)
            sc_sb = pool.tile([128, 2], F32, tag="sc_sb")
            nc.vector.tensor_copy(sc_sb[:, :], psum_sc[:, :])
            psum_sfbc = psb.tile([128, 32], F32, tag="psum_sfbc")
            nc.tensor.matmul(psum_sfbc[:, :], gr8_v, pool2sf[:, :],
                             start=True, stop=True)
            sfterm = pool.tile([128, 32], F32, tag="sfterm")
            nc.vector.tensor_scalar(sfterm[:, :], psum_sfbc[:, :],
                                    1.0 / 64, sc_sb[:, 0:1], OP.mult, OP.subtract)
            nc.vector.tensor_scalar_mul(sfterm[:, :], sfterm[:, :], sc_sb[:, 1:2])
            flat_f = pool.tile([128, 32], F32, tag="flat_f")
            nc.vector.tensor_scalar_mul(flat_f[:, :], pool2[:, :], 1.0 / 64)
            nc.vector.tensor_tensor(flat_f[:, :], flat_f[:, :], sfterm[:, :], OP.add)
            flat_bf = pool.tile([128, 32], BF16, tag="flat_bf")
            nc.vector.tensor_copy(flat_bf[:, :], flat_f[:, :])

            # MLP
            psum_h = psb.tile([1, 128], F32, tag="psum_h")
            for j in range(32):
                nc.tensor.matmul(psum_h[:, :], flat_bf[:, j:j + 1],
                                 w1_bf[:, j, :], start=(j == 0), stop=(j == 31))
            hr = pool.tile([1, 128], F32, tag="hr")
            nc.scalar.activation(hr[:, :], psum_h[:, :], AF.Relu)
            hw2 = pool.tile([1, 128], F32, tag="hw2")
            nc.vector.tensor_tensor(hw2[:, :], hr[:, :], w2_v, OP.mult)
            dsc = pool.tile([1, 4], F32, tag="dsc")
            nc.vector.tensor_reduce(dsc[:, 0:1], hw2[:, :], AX.X, OP.add)
            nc.scalar.activation(dsc[:, 1:2], dsc[:, 0:1], AF.Sigmoid)
            nc.vector.tensor_scalar_mul(dsc[:, 2:3], dsc[:, 1:2], float(MD))
            nc.vector.reciprocal(dsc[:, 3:4], dsc[:, 2:3])

            # w' = exp(-dist/d)/sum (+1 center)
            psum_i11 = psb.tile([11, 1], F32, tag="psum_i11")
            nc.tensor.matmul(psum_i11[:, :], o111_v, dsc[:, 3:4],
                             start=True, stop=True)
            invd_col = pool.tile([11, 1], F32, tag="invd_col")
            nc.vector.tensor_copy(invd_col[:, :], psum_i11[:, :])
            wexp = pool.tile([11, 11], F32, tag="wexp")
            nc.scalar.activation(wexp[:, :], dnt_v, AF.Exp,
                                 scale=invd_col[:, 0:1])
            psum_ws = psb.tile([1, 11], F32, tag="psum_ws")
            nc.tensor.matmul(psum_ws[:, :], o11_v, wexp[:, :],
                             start=True, stop=True)
            wsv = pool.tile([1, 2], F32, tag="wsv")
            nc.vector.tensor_reduce(wsv[:, 0:1], psum_ws[:, :], AX.X, OP.add)
            nc.vector.reciprocal(wsv[:, 1:2], wsv[:, 0:1])
            psum_w11 = psb.tile([11, 1], F32, tag="psum_w11")
            nc.tensor.matmul(psum_w11[:, :], o111_v, wsv[:, 1:2],
                             start=True, stop=True)
            wsi_col = pool.tile([11, 1], F32, tag="wsi_col")
            nc.vector.tensor_copy(wsi_col[:, :], psum_w11[:, :])
            wp_sb = pool.tile([11, 400], F32, tag="wp_sb")
            nc.vector.memset(wp_sb[:, :], 0.0)
            nc.vector.tensor_scalar_mul(wp_sb[:, 250:261], wexp[:, :],
                                        wsi_col[:, 0:1])
            nc.vector.tensor_tensor(wp_sb[:, 250:261], wp_sb[:, 250:261],
                                    cm_v, OP.add)
            nc.sync.dma_start(out=vdram[:, :], in_=wp_sb[:, :])

            # T matrices via sliding-window DMA + fold
            t_ext = pool.tile([128, 11, 138], F32, tag="t_ext")
            # Load T with all-positive steps (contiguous 552B runs) by
            # storing j REVERSED: t_ext[p, dy, jr] = v[dy, 123 + p + jr]
            # (valid because the kernel rows are symmetric in dx, so the
            # reversed generator equals the original). xe = 132 - jr.
            nc.gpsimd.dma_start(
                out=t_ext[:, :, :],
                in_=AP(tensor=vdram, offset=123,
                       ap=[[1, 128], [400, 11], [1, 138]]))
            # folds in reversed coords: xe=0 target at jr=132 (sources
            # jr 133..137), xe=127 target at jr=5 (sources jr 0..4)
            tl = pool.tile([128, 11, 1], F32, tag="tl")
            th = pool.tile([128, 11, 1], F32, tag="th")
            nc.vector.tensor_reduce(tl[:, :, :], t_ext[:, :, 133:138], AX.X, OP.add)
            nc.vector.tensor_reduce(th[:, :, :], t_ext[:, :, 0:5], AX.X, OP.add)
            nc.vector.tensor_tensor(t_ext[:, :, 132:133], t_ext[:, :, 132:133],
                                    tl[:, :, :], OP.add)
            nc.vector.tensor_tensor(t_ext[:, :, 5:6], t_ext[:, :, 5:6],
                                    th[:, :, :], OP.add)
            t_fold = pool.tile([128, 11, 128], BF16, tag="t_fold")
            tea = t_ext[:, :, :]
            nc.vector.tensor_copy(
                t_fold[:, :, :],
                AP(tensor=tea.tensor, offset=tea.offset + 132,
                   ap=[list(tea.ap[0]), [138, 11], [-1, 128]]))

            # ============== P merge (after scatter chains land) ==============
            def pt_dst(t0):
                # [128, 64 slots step 2, 64] view into p_t starting at slot t0
                a = p_t[:, :]
                return AP(tensor=a.tensor, offset=a.offset + t0 * 64,
                          ap=[list(a.ap[0]), [128, 64], [1, 64]])

            nc.vector.tensor_tensor(
                pt_dst(PT_OFF),
                pbuf[0][:, 0:4096].rearrange("p (a b) -> p a b", b=64),
                pbuf[2][:, 0:4096].rearrange("p (a b) -> p a b", b=64), OP.add)
            nc.vector.tensor_tensor(
                pt_dst(PT_OFF + 1),
                pbuf[1][:, 0:4096].rearrange("p (a b) -> p a b", b=64),
                pbuf[3][:, 0:4096].rearrange("p (a b) -> p a b", b=64), OP.add)

            # ============== Phase C: Toeplitz conv + tail ==============
            psum_stack.close()
            psum_stack = ExitStack()
            psc = psum_stack.enter_context(
                tc.tile_pool(name="psc", bufs=2, space="PSUM"))
            ft = pool.tile([128, 8192], BF16, tag="ft")

            p_t_flat = p_t[:, :]
            for ps in range(5):
                nslots = 32 if ps < 4 else 16
                psum_c = psc.tile([128, nslots * 64], F32, tag="psum_c")
                for dyi in range(11):
                    dy = dyi - 5
                    for ch in range(nslots // 8):
                        u0 = 32 * ps + 8 * ch
                        off = (u0 + 8 - dy) * 64
                        rhs = p_t_flat[:, off:off + 512].rearrange("p a -> p a")
                        nc.tensor.matmul(
                            psum_c[:, ch * 512:ch * 512 + 512],
                            t_fold[:, dyi, :], rhs,
                            start=(dyi == 0), stop=(dyi == 10))
                # tail: feat_T += psum slice  (real y rows of this pass)
                y0 = max(0, 32 * ps - 8)
                y1 = min(128, 32 * ps + 24)
                nc.sync.dma_start(out=ft[:, y0 * 64:y1 * 64],
                                  in_=feat_T[:, y0 * 64:y1 * 64])
                po = (y0 + 8 - 32 * ps) * 64
                nc.vector.tensor_tensor(
                    ft[:, y0 * 64:y1 * 64],
                    psum_c[:, po:po + (y1 - y0) * 64],
                    ft[:, y0 * 64:y1 * 64], OP.add)
                if ps == 0:
                    pa = psum_c[:, :]
                    tmpf = pool.tile([128, 64], F32, tag="tmpf")
                    nc.vector.tensor_reduce(
                        tmpf[:, :],
                        AP(tensor=pa.tensor, offset=pa.offset + 3 * 64,
                           ap=[list(pa.ap[0]), [1, 64], [64, 5]]),
                        AX.X, OP.add)
                    nc.vector.tensor_tensor(ft[:, 0:64], ft[:, 0:64],
                                            tmpf[:, :], OP.add)
                if ps == 4:
                    pa = psum_c[:, :]
                    tmph = pool.tile([128, 64], F32, tag="tmph")
                    nc.vector.tensor_reduce(
                        tmph[:, :],
                        AP(tensor=pa.tensor, offset=pa.offset + 8 * 64,
                           ap=[list(pa.ap[0]), [1, 64], [64, 5]]),
                        AX.X, OP.add)
                    nc.vector.tensor_tensor(ft[:, 127 * 64:128 * 64],
                                            ft[:, 127 * 64:128 * 64],
                                            tmph[:, :], OP.add)
                nc.sync.dma_start(out=out_T[:, y0 * 64:y1 * 64],
                                  in_=ft[:, y0 * 64:y1 * 64])

            psum_stack.close()

    nc.compile()
    return nc


def build_core_inputs(x, xyz, feature, conv_w, conv_b, gn_gamma, gn_beta,
                      mlp_w1, mlp_w2):
    """Host-side sharding glue: slice batch b per core + layout transforms."""
    import ml_dtypes
    f32 = np.float32
    bf16 = ml_dtypes.bfloat16
    # shared constants
    convw_pl = np.zeros((128, 4, 128), f32)
    sfw_pl = np.zeros((128, 4, 8), f32)
    for h in range(2):
        for g in range(4):
            convw_pl[64 * h:64 * h + 64, g, np.arange(16) * 8 + h * 4 + g] = conv_w.T
            sfw_pl[64 * h:64 * h + 64, g, h * 4 + g] = 1.0
    gmat = (np.arange(128)[:, None] // 32 == np.arange(4)[None, :]).astype(f32)
    ltri_m = (np.arange(128)[:, None] < np.arange(128)[None, :]).astype(f32)
    trash_m = (16384 + (np.arange(32)[None, :] % 2) * 128
               + np.arange(128)[:, None]).astype(f32)
    ident_m = np.eye(128, dtype=f32)
    selw_m = np.zeros((128, 128), f32)
    for g in range(8):
        for q in range(16):
            selw_m[16 * g + q, g * 16 + q] = 1.0
    blob1 = np.zeros((128, 968), f32)
    blob1[:, 0:128] = ltri_m
    blob1[:, 128:256] = ident_m
    blob1[:, 256:384] = selw_m
    blob1[:, 384:896] = convw_pl.reshape(128, 512)
    blob1[:, 896:928] = trash_m
    blob1[:, 928:960] = sfw_pl.reshape(128, 32)
    blob1[:, 960:964] = gmat
    blob1[:, 964] = np.repeat(conv_b, 8)
    blob1[:, 965] = np.repeat(gn_gamma, 8)
    blob1[:, 966] = np.repeat(gn_beta, 8)
    blob1[:, 967] = 1.0
    dxy = np.arange(11) - 5
    blob2 = np.zeros((11, 546), f32)
    blob2[0:4, 0:128] = gmat.T
    blob2[0:8, 128:256] = (np.arange(128)[None, :] % 8
                           == np.arange(8)[:, None]).astype(f32)
    blob2[0, 256:384] = mlp_w2[0]
    blob2[0, 384:512] = 1.0
    blob2[0:11, 512:523] = -np.sqrt(dxy[None, :] ** 2 + dxy[:, None] ** 2)
    blob2[5, 523 + 5] = 1.0
    blob2[0:11, 534] = 1.0
    blob2[0, 535:546] = 1.0
    # w1 rearranged: w1n[p=(o,seg), j=(by_l,bx), n] = mlp_w1[n, o*256+(seg*2+by_l)*16+bx]
    o = np.arange(16)[:, None, None, None]
    seg = np.arange(8)[None, :, None, None]
    byl = np.arange(2)[None, None, :, None]
    bx = np.arange(16)[None, None, None, :]
    fl = (o * 256 + (seg * 2 + byl) * 16 + bx).reshape(128, 32)
    w1n = np.ascontiguousarray(mlp_w1.T[fl]).astype(bf16)  # [128, 32, 128]
    shared = dict(blob1=blob1, blob2=blob2, w1n=w1n)

    in_maps = []
    for b in range(B):
        fb = np.ascontiguousarray(feature[b].reshape(64, 16384)).astype(bf16)
        m = dict(shared)
        m["x_wrap"] = np.ascontiguousarray(
            x[b].reshape(32, 128, 64).transpose(1, 0, 2)).astype(bf16)
        m["xx_w2"] = np.ascontiguousarray(xyz[b, :, 0].reshape(32, 128).T).astype(f32)
        m["xy_w2"] = np.ascontiguousarray(xyz[b, :, 1].reshape(32, 128).T).astype(f32)
        m["feat_nat2"] = np.concatenate([fb[:, :8192], fb[:, 8192:]], axis=0)
        m["feat_T"] = np.ascontiguousarray(
            feature[b].transpose(2, 1, 0).reshape(128, 8192)).astype(bf16)
        in_maps.append(m)
    return in_maps


_NC_CACHE = {}


def kernel(x, xyz, feature, conv_w, conv_b, gn_gamma, gn_beta, mlp_w1, mlp_w2,
           _trace=False):
    from concourse.bass_utils import run_bass_kernel_spmd
    if "nc" not in _NC_CACHE:
        _NC_CACHE["nc"] = build_nc()
    nc = _NC_CACHE["nc"]
    in_maps = build_core_inputs(np.asarray(x), np.asarray(xyz),
                                np.asarray(feature), np.asarray(conv_w),
                                np.asarray(conv_b), np.asarray(gn_gamma),
                                np.asarray(gn_beta), np.asarray(mlp_w1),
                                np.asarray(mlp_w2))
    res = run_bass_kernel_spmd(nc, in_maps, core_ids=list(range(8)),
                               trace=_trace)
    outs = []
    for i in range(B):
        ot = np.asarray(res.results[i]["out_T"]).astype(np.float32)
        outs.append(ot.reshape(128, 128, 64).transpose(2, 1, 0))
    out = np.stack(outs).astype(np.float32)
    if _trace:
        return out, res
    return out


# revision 12
# speedup vs baseline: 1.0906x; 1.0832x over previous
"""Trainium2 Bass kernel for nn_AMM_w_AFDM (scatter_memory).

Strategy (one batch per NeuronCore, 8 cores data-parallel):
  out[b] = feature + P + splat(P, w)  where P = nearest-cell scatter of x.
  The 11x11 splat-with-border-clipping is computed as a bank of banded
  Toeplitz matmuls on TensorE; P is built with gpsimd.dma_scatter_add
  (SBUF parity-split destination), pipelined per-half with the dedup so
  the gpsimd descriptor generation overlaps the dedup math. The AFDM
  dilation scalar is computed on-device and overlaps the scatter window.
  Inputs feature/x/w1 are pre-cast to bf16 host-side; output is bf16.
"""
import sys
from contextlib import ExitStack
import numpy as np

sys.path.insert(0, "/opt/trn_rl_repo")

import concourse.bacc as bacc  # noqa: E402
import concourse.bass as bass  # noqa: E402
import concourse.mybir as mybir  # noqa: E402
import concourse.tile as tile  # noqa: E402
from concourse.ap import AP  # noqa: E402

MD = 5
EPS = 1e-5
B, N, C, H, W = 8, 4096, 64, 128, 128
F32 = mybir.dt.float32
BF16 = mybir.dt.bfloat16
I16 = mybir.dt.int16
AX = mybir.AxisListType
OP = mybir.AluOpType
AF = mybir.ActivationFunctionType

# P_T slot layout: slot t in [0,160), y = t-16 (zeros outside [0,128)).
PT_SLOTS = 160
PT_OFF = 16
# ext output slots: u in [0,144), ye = u-8.
EXT_SLOTS = 144


def build_nc():
    nc = bacc.Bacc("TRN2", target_bir_lowering=False)

    def din(name, shape, dt=F32):
        return nc.dram_tensor(name, shape, dt, kind="ExternalInput")

    x_wrap = din("x_wrap", [128, 32, 64], BF16)
    xx_w2 = din("xx_w2", [128, 32])
    xy_w2 = din("xy_w2", [128, 32])
    feat_nat2 = din("feat_nat2", [128, 8192], BF16)
    feat_T = din("feat_T", [128, 8192], BF16)
    blob1 = din("blob1", [128, 968])
    blob2 = din("blob2", [11, 546])
    w1n = din("w1n", [128, 32, 128], BF16)
    lint_dram = nc.dram_tensor("lint_dram", [32, 128], F32)

    out_T = nc.dram_tensor("out_T", [128, 8192], BF16, kind="ExternalOutput")
    vdram = nc.dram_tensor("vdram", [11, 400], F32)

    with tile.TileContext(nc) as tc:
        with tc.tile_pool(name="main", bufs=1) as pool, \
             tc.tile_pool(name="scat", bufs=1) as scpool, \
             tc.tile_pool(name="stage", bufs=2) as stpool:
            psum_stack = ExitStack()

            # ============== early input DMAs ==============
            ps_prep = psum_stack.enter_context(
                tc.tile_pool(name="psp", bufs=2, space="PSUM"))
            xx2 = pool.tile([128, 32], F32, tag="xx2")
            xy2 = pool.tile([128, 32], F32, tag="xy2")
            nc.sync.dma_start(out=xx2[:, :], in_=xx_w2[:, :])
            nc.sync.dma_start(out=xy2[:, :], in_=xy_w2[:, :])
            blob1_sb = pool.tile([128, 968], F32, tag="blob1_sb")
            nc.sync.dma_start(out=blob1_sb[:, :], in_=blob1[:, :])
            blob2_sb = pool.tile([11, 546], F32, tag="blob2_sb")
            nc.sync.dma_start(out=blob2_sb[:, :], in_=blob2[:, :])
            x_bf = scpool.tile([128, 32, 64], BF16, tag="x_bf")
            nc.sync.dma_start(out=x_bf[:, :, :], in_=x_wrap[:, :, :])
            fnat_bf = pool.tile([128, 8192], BF16, tag="fnat_bf")
            nc.sync.dma_start(out=fnat_bf[:, :], in_=feat_nat2[:, :])
            w1_bf = pool.tile([128, 32, 128], BF16, tag="w1_bf")
            nc.sync.dma_start(out=w1_bf[:, :, :], in_=w1n[:, :, :])

            # ============== Phase S: scatter x -> P ==============
            # Index math in the 128-wrap: lin128[p, t] for point j = t*128+p.
            # (emitted FIRST on the DVE queue so the dedup pipeline starts
            #  immediately; buffer memsets follow on gpsimd/DVE)
            def floor127(srcw, sfx):
                # floor(t) = round(t) - (round(t) > t), round via +/- 2^23
                t = pool.tile([128, 32], F32, tag=f"fl_t{sfx}")
                r = pool.tile([128, 32], F32, tag=f"fl_r{sfx}")
                g = pool.tile([128, 32], F32, tag=f"fl_g{sfx}")
                o = pool.tile([128, 32], F32, tag=f"fl_o{sfx}")
                nc.vector.tensor_scalar_mul(t[:, :], srcw[:, :], 127.0)
                nc.vector.tensor_scalar(r[:, :], t[:, :], 8388608.0, -8388608.0,
                                        OP.add, OP.add)
                nc.vector.tensor_tensor(g[:, :], r[:, :], t[:, :], OP.is_gt)
                nc.vector.tensor_tensor(o[:, :], r[:, :], g[:, :], OP.subtract)
                return o

            xc = floor127(xx2, "x")
            yc = floor127(xy2, "y")
            lin128 = pool.tile([128, 32], F32, tag="lin128")
            nc.vector.tensor_scalar_mul(lin128[:, :], yc[:, :], 128.0)
            nc.vector.tensor_tensor(lin128[:, :], lin128[:, :], xc[:, :], OP.add)

            ltri_f = blob1_sb[:, 0:128]
            ident_v = blob1_sb[:, 128:256]
            selw_v = blob1_sb[:, 256:384]
            cw_view = blob1_sb[:, 384:896].rearrange("p (a b) -> p a b", b=128)
            trash_sb = blob1_sb[:, 896:928]
            sw_view = blob1_sb[:, 928:960].rearrange("p (a b) -> p a b", b=8)
            gm_v = blob1_sb[:, 960:964]
            cb_col = blob1_sb[:, 964:965]
            gg_col = blob1_sb[:, 965:966]
            gb_col = blob1_sb[:, 966:967]
            onesc_f = blob1_sb[:, 967:968]
            gmt_v = blob2_sb[0:4, 0:128]
            gr8_v = blob2_sb[0:8, 128:256]
            w2_v = blob2_sb[0:1, 256:384]
            o1128_v = blob2_sb[0:1, 384:512]
            dnt_v = blob2_sb[0:11, 512:523]
            cm_v = blob2_sb[0:11, 523:534]
            o11_v = blob2_sb[0:11, 534:535]
            o111_v = blob2_sb[0:1, 535:546]
            ltri_sb = pool.tile([128, 128], BF16, tag="ltri_sb")
            nc.vector.tensor_copy(ltri_sb[:, :], ltri_f)
            onesc_bf = pool.tile([128, 1], BF16, tag="onesc_bf")
            nc.vector.tensor_copy(onesc_bf[:, :], onesc_f)
            ident_bf = pool.tile([128, 128], BF16, tag="ident_bf")
            nc.vector.tensor_copy(ident_bf[:, :], ident_v)

            xm = scpool.tile([128, 32, 64], BF16, tag="xm")

            # scatter destination buffers: memsets on gpsimd, which is idle
            # until the first scatter call anyway — keeps DVE free for dedup
            pb0 = scpool.tile([128, 66 * 64], BF16, tag="pb0")
            pb1 = scpool.tile([128, 66 * 64], BF16, tag="pb1")
            pb2 = scpool.tile([128, 66 * 64], BF16, tag="pb2")
            pb3 = scpool.tile([128, 66 * 64], BF16, tag="pb3")
            pbuf = [pb0, pb1, pb2, pb3]  # [A_even, A_odd, B_even, B_odd]
            for pb in pbuf:
                nc.gpsimd.memset(pb[:, 0:4096], 0.0)
            # P accumulators (bf16 for the exact identity term, fp8 for the
            # DoubleRow Toeplitz conv); border memsets emitted after dedup
            p_t = pool.tile([128, PT_SLOTS * 64], BF16, tag="p_t")
            p8 = pool.tile([128, PT_SLOTS * 64], mybir.dt.float8e4, tag="p8")

            # Per-256-chunk dedup: chunk a = slots (2a, 2a+1) = subs (u, v).
            first128 = pool.tile([128, 32], F32, tag="first128")
            ps_lt = ps_prep.tile([32, 128], F32, tag="ps_lt")
            nc.tensor.transpose(ps_lt[:, :], lin128[:, :], ident_v)
            linT = pool.tile([32, 128], F32, tag="linT")
            nc.vector.tensor_copy(linT[:, :], ps_lt[:, :])
            nc.sync.dma_start(out=lint_dram[:, :], in_=linT[:, :])

            def eq_mat(out_bf, bc_psum, bc_off, col_t):
                # out[q, p] = (lin(col_t, q) == bcast[bc_off + p])
                nc.vector.tensor_scalar(out_bf[:, :], bc_psum[:, bc_off:bc_off + 128],
                                        lin128[:, col_t:col_t + 1], None, OP.is_equal)

            idxf = pool.tile([128, 32], F32, tag="idxf")
            idx_sb = pool.tile([128, 256], I16, tag="idx")
            idxw_f = pool.tile([16, 256], F32, tag="idxw_f")

            for hf in range(2):
                for a4 in range(4 * hf, 4 * hf + 4):  # 4 slots per bcast matmul
                    lrow = stpool.tile([1, 512], F32, tag="lrow")
                    nc.sync.dma_start(
                        out=lrow[:, :],
                        in_=AP(tensor=lint_dram, offset=a4 * 512,
                               ap=[[1, 1], [1, 512]]))
                    bc = ps_prep.tile([128, 512], F32, tag="bc")
                    nc.tensor.matmul(bc[:, :], o1128_v, lrow[:, :],
                                     start=True, stop=True)
                    for ci in range(2):
                        a = a4 * 2 + ci
                        u, v = 2 * a, 2 * a + 1
                        uoff, voff = (u % 4) * 128, (v % 4) * 128
                        m_uu = pool.tile([128, 128], BF16, tag="m_uu")
                        m_vv = pool.tile([128, 128], BF16, tag="m_vv")
                        m_uv = pool.tile([128, 128], BF16, tag="m_uv")
                        m_vu = pool.tile([128, 128], BF16, tag="m_vu")
                        eq_mat(m_uu, bc, uoff, u)   # rows q: sub u, cols p: sub u
                        eq_mat(m_vv, bc, voff, v)   # rows: v, cols: v
                        eq_mat(m_uv, bc, voff, u)   # rows: u, cols: v
                        eq_mat(m_vu, bc, uoff, v)   # rows: v, cols: u
                        # merged_u = m_uu.T @ x_u + m_vu.T @ x_v
                        pm = ps_prep.tile([128, 128], F32, tag="pm")
                        nc.tensor.matmul(pm[:, 0:64], m_uu[:, :], x_bf[:, u, :],
                                         start=True, stop=False)
                        nc.tensor.matmul(pm[:, 0:64], m_vu[:, :], x_bf[:, v, :],
                                         start=False, stop=True)
                        nc.tensor.matmul(pm[:, 64:128], m_vv[:, :], x_bf[:, v, :],
                                         start=True, stop=True)
                        nc.scalar.activation(xm[:, u:u + 2, :], pm[:, :], AF.Copy)
                        # counts: below_u = (m_uu*L).T @ 1 ; below_v = (m_vv*L).T @ 1
                        #         + m_uv.T @ 1 (any match in sub u)
                        ml_u = pool.tile([128, 128], BF16, tag="ml_u")
                        ml_v = pool.tile([128, 128], BF16, tag="ml_v")
                        nc.vector.tensor_tensor(ml_u[:, :], m_uu[:, :], ltri_sb[:, :],
                                                OP.mult)
                        nc.vector.tensor_tensor(ml_v[:, :], m_vv[:, :], ltri_sb[:, :],
                                                OP.mult)
                        cnt = ps_prep.tile([128, 2], F32, tag="cnt")
                        nc.tensor.matmul(cnt[:, 0:1], ml_u[:, :], onesc_bf[:, :],
                                         start=True, stop=True)
                        nc.tensor.matmul(cnt[:, 1:2], ml_v[:, :], onesc_bf[:, :],
                                         start=True, stop=False)
                        nc.tensor.matmul(cnt[:, 1:2], m_uv[:, :], onesc_bf[:, :],
                                         start=False, stop=True)
                        nc.vector.tensor_scalar(first128[:, u:u + 2], cnt[:, :],
                                                0.5, None, OP.is_lt)

                # idx = first ? lin : trash for this half, then scatter chain
                hs = slice(16 * hf, 16 * hf + 16)
                nc.vector.tensor_tensor(idxf[:, hs], lin128[:, hs],
                                        trash_sb[:, hs], OP.subtract)
                nc.vector.tensor_tensor(idxf[:, hs], idxf[:, hs],
                                        first128[:, hs], OP.mult)
                nc.vector.tensor_tensor(idxf[:, hs], idxf[:, hs],
                                        trash_sb[:, hs], OP.add)
                # 16-wrap: idxw[q, t*8+g] = idxf[16g+q, t]
                for g in range(8):
                    ps_w = ps_prep.tile([16, 16], F32, tag="bc")
                    nc.tensor.matmul(ps_w[:, :], selw_v[:, 16 * g:16 * g + 16],
                                     idxf[:, hs], start=True, stop=True)
                    dst = idxw_f[:, :]
                    nc.vector.tensor_copy(
                        AP(tensor=dst.tensor, offset=dst.offset + 128 * hf + g,
                           ap=[list(dst.ap[0]), [8, 16]]),
                        ps_w[:, :])
                nc.vector.tensor_copy(idx_sb[0:16, 128 * hf:128 * hf + 128],
                                      idxw_f[:, 128 * hf:128 * hf + 128])
                for r in range(1, 8):
                    nc.sync.dma_start(
                        out=idx_sb[16 * r:16 * r + 16, 128 * hf:128 * hf + 128],
                        in_=idx_sb[0:16, 128 * hf:128 * hf + 128])

                # scatter this half's 8 chunks; alternate chains for DMA overlap
                for a in range(8 * hf, 8 * hf + 8):
                    ch = a % 2
                    nc.gpsimd.dma_scatter_add(
                        pbuf[2 * ch][:, :], xm[:, 2 * a:2 * a + 2, :],
                        idx_sb[:, a * 16:(a + 1) * 16],
                        256, 256, 64,
                        parity_reg=0, out_ap_other=pbuf[2 * ch + 1][:, :],
                        sbuf_tokens_per_rank=128,
                    )

            # P border slots (small; DVE reaches these right after dedup)
            nc.vector.memset(p_t[:, 0:PT_OFF * 64], 0.0)
            nc.vector.memset(p_t[:, (PT_OFF + 128) * 64:], 0.0)
            nc.vector.memset(p8[:, 0:PT_OFF * 64], 0.0)
            nc.vector.memset(p8[:, (PT_OFF + 128) * 64:], 0.0)

            psum_stack.close()
            psum_stack = ExitStack()
            psa = psum_stack.enter_context(
                tc.tile_pool(name="psa", bufs=1, space="PSUM"))

            # ============== Phase A: AFDM dilation ==============
            # (emitted before the P merge so its DVE/ACT/PE ops run during the
            #  scatter window)
            cw_bf = pool.tile([128, 4, 128], BF16, tag="cw_bf")
            sw_bf = pool.tile([128, 4, 8], BF16, tag="sw_bf")
            nc.vector.tensor_copy(cw_bf[:, :, :], cw_view)
            nc.vector.tensor_copy(sw_bf[:, :, :], sw_view)

            # conv1x1 (16 o-ch x 8 seg packed in M=128) + sf sums, 16 windows
            psum_r = psa.tile([128, 2048], F32, tag="psum_r")
            psum_sf = psa.tile([8, 2048], F32, tag="psum_sf")
            for wnd in range(16):
                g = wnd // 4
                st_flag = wnd < 4
                sp_flag = wnd >= 12
                rhs = fnat_bf[:, wnd * 512:(wnd + 1) * 512]
                nc.tensor.matmul(psum_r[:, (wnd % 4) * 512:(wnd % 4) * 512 + 512],
                                 cw_bf[:, g, :], rhs, start=st_flag, stop=sp_flag)
                nc.tensor.matmul(psum_sf[:, (wnd % 4) * 512:(wnd % 4) * 512 + 512],
                                 sw_bf[:, g, :], rhs, start=st_flag, stop=sp_flag)

            # r = psum_r + conv_b  (ACT Identity with bias col)
            r_sb = pool.tile([128, 2048], F32, tag="r_sb")
            nc.scalar.activation(r_sb[:, :], psum_r[:, :], AF.Identity,
                                 bias=cb_col[:, 0:1])
            # sf: min/max + pool
            sfmm = pool.tile([8, 2], F32, tag="sfmm")
            nc.vector.tensor_reduce(sfmm[:, 0:1], psum_sf[:, :], AX.X, OP.min)
            nc.vector.tensor_reduce(sfmm[:, 1:2], psum_sf[:, :], AX.X, OP.max)
            pool1sf = pool.tile([8, 256], F32, tag="pool1sf")
            nc.vector.tensor_reduce(pool1sf[:, :],
                                    psum_sf[:, :].rearrange("p (a b) -> p a b", b=8),
                                    AX.X, OP.add)
            pool2sf = pool.tile([8, 32], F32, tag="pool2sf")
            p1s = pool1sf[:, :]
            nc.vector.tensor_reduce(
                pool2sf[:, :].rearrange("p (a b) -> p a b", b=16),
                AP(tensor=p1s.tensor, offset=p1s.offset,
                   ap=[list(p1s.ap[0]), [128, 2], [1, 16], [16, 8]]),
                AX.X, OP.add)

            # close psa (psum_r / psum_sf fully consumed), open psb
            psum_stack.close()
            psum_stack = ExitStack()
            psb = psum_stack.enter_context(
                tc.tile_pool(name="psb", bufs=1, space="PSUM"))
            # stats: s1 = sum r, s2 = sum r^2 (ACT Square w/ accum)
            s_col = pool.tile([128, 2], F32, tag="s_col")
            nc.vector.tensor_reduce(s_col[:, 0:1], r_sb[:, :], AX.X, OP.add)
            rr_sb = pool.tile([128, 2048], BF16, tag="rr_sb")
            nc.scalar.activation(rr_sb[:, :], r_sb[:, :], AF.Square,
                                 accum_out=s_col[:, 1:2])
            psum_st = psb.tile([4, 2], F32, tag="psum_st")
            nc.tensor.matmul(psum_st[:, :], gm_v, s_col[:, :],
                             start=True, stop=True)
            # mu, rstd
            stt = pool.tile([4, 6], F32, tag="stt")
            inv_cnt = 1.0 / (4 * H * W)
            nc.vector.tensor_scalar_mul(stt[:, 0:1], psum_st[:, 0:1], inv_cnt)
            nc.vector.tensor_scalar_mul(stt[:, 1:2], psum_st[:, 1:2], inv_cnt)
            nc.vector.tensor_tensor(stt[:, 2:3], stt[:, 0:1], stt[:, 0:1], OP.mult)
            nc.vector.tensor_tensor(stt[:, 3:4], stt[:, 1:2], stt[:, 2:3], OP.subtract)
            nc.vector.tensor_scalar_add(stt[:, 3:4], stt[:, 3:4], EPS)
            nc.scalar.activation(stt[:, 4:5], stt[:, 3:4], AF.Sqrt)
            nc.vector.reciprocal(stt[:, 5:6], stt[:, 4:5])
            bc_in = pool.tile([4, 2], F32, tag="bc_in")
            nc.vector.tensor_copy(bc_in[:, 0:1], stt[:, 0:1])
            nc.vector.tensor_copy(bc_in[:, 1:2], stt[:, 5:6])
            psum_bc = psb.tile([128, 2], F32, tag="psum_bc")
            nc.tensor.matmul(psum_bc[:, :], gmt_v, bc_in[:, :],
                             start=True, stop=True)
            a_col = pool.tile([128, 1], F32, tag="a_col")
            b_col = pool.tile([128, 1], F32, tag="b_col")
            nc.vector.tensor_tensor(a_col[:, :], psum_bc[:, 1:2], gg_col[:, :], OP.mult)
            nc.vector.tensor_tensor(b_col[:, :], psum_bc[:, 0:1], a_col[:, :], OP.mult)
            nc.vector.tensor_tensor(b_col[:, :], gb_col[:, :], b_col[:, :], OP.subtract)
            # rr = relu(r*a + b)
            nc.scalar.activation(rr_sb[:, :], r_sb[:, :], AF.Relu,
                                 bias=b_col[:, 0:1], scale=a_col[:, 0:1])
            # pool 8x8
            pool1 = pool.tile([128, 256], F32, tag="pool1")
            nc.vector.tensor_reduce(pool1[:, :],
                                    rr_sb[:, :].rearrange("p (a b) -> p a b", b=8),
                                    AX.X, OP.add)
            pool2 = pool.tile([128, 32], F32, tag="pool2")
            p1a = pool1[:, :]
            nc.vector.tensor_reduce(
                pool2[:, :].rearrange("p (a b) -> p a b", b=16),
                AP(tensor=p1a.tensor, offset=p1a.offset,
                   ap=[list(p1a.ap[0]), [128, 2], [1, 16], [16, 8]]),
                AX.X, OP.add)

            mm2t = pool.tile([1, 16], F32, tag="mm2t")
            nc.sync.dma_start(out=mm2t[:, :], in_=sfmm[:, :])
            gmn = pool.tile([1, 4], F32, tag="gmn")
            mma = mm2t[:, :]
            nc.vector.tensor_reduce(
                gmn[:, 0:1],
                AP(tensor=mma.tensor, offset=mma.offset, ap=[list(mma.ap[0]), [2, 8]]),
                AX.X, OP.min)
            nc.vector.tensor_reduce(
                gmn[:, 1:2],
                AP(tensor=mma.tensor, offset=mma.offset + 1,
                   ap=[list(mma.ap[0]), [2, 8]]),
                AX.X, OP.max)
            nc.vector.tensor_tensor(gmn[:, 2:3], gmn[:, 1:2], gmn[:, 0:1], OP.subtract)
            nc.vector.tensor_scalar_add(gmn[:, 2:3], gmn[:, 2:3], EPS)
            nc.vector.reciprocal(gmn[:, 3:4], gmn[:, 2:3])
            pack12 = pool.tile([1, 2], F32, tag="pack12")
            nc.vector.tensor_copy(pack12[:, 0:1], gmn[:, 0:1])
            nc.vector.tensor_copy(pack12[:, 1:2], gmn[:, 3:4])
            psum_sc = psb.tile([128, 2], F32, tag="psum_sc")
            nc.tensor.matmul(psum_sc[:, :], o1128_v, pack12[:, :],
                             start=True, stop=True)
            sc_sb = pool.tile([128, 2], F32, tag="sc_sb")
            nc.vector.tensor_copy(sc_sb[:, :], psum_sc[:, :])
            psum_sfbc = psb.tile([128, 32], F32, tag="psum_sfbc")
            nc.tensor.matmul(psum_sfbc[:, :], gr8_v, pool2sf[:, :],
                             start=True, stop=True)
            sfterm = pool.tile([128, 32], F32, tag="sfterm")
            nc.vector.tensor_scalar(sfterm[:, :], psum_sfbc[:, :],
                                    1.0 / 64, sc_sb[:, 0:1], OP.mult, OP.subtract)
            nc.vector.tensor_scalar_mul(sfterm[:, :], sfterm[:, :], sc_sb[:, 1:2])
            flat_f = pool.tile([128, 32], F32, tag="flat_f")
            nc.vector.tensor_scalar_mul(flat_f[:, :], pool2[:, :], 1.0 / 64)
            nc.vector.tensor_tensor(flat_f[:, :], flat_f[:, :], sfterm[:, :], OP.add)
            flat_bf = pool.tile([128, 32], BF16, tag="flat_bf")
            nc.vector.tensor_copy(flat_bf[:, :], flat_f[:, :])

            # MLP
            psum_h = psb.tile([1, 128], F32, tag="psum_h")
            for j in range(32):
                nc.tensor.matmul(psum_h[:, :], flat_bf[:, j:j + 1],
                                 w1_bf[:, j, :], start=(j == 0), stop=(j == 31))
            hr = pool.tile([1, 128], F32, tag="hr")
            nc.scalar.activation(hr[:, :], psum_h[:, :], AF.Relu)
            hw2 = pool.tile([1, 128], F32, tag="hw2")
            nc.vector.tensor_tensor(hw2[:, :], hr[:, :], w2_v, OP.mult)
            dsc = pool.tile([1, 4], F32, tag="dsc")
            nc.vector.tensor_reduce(dsc[:, 0:1], hw2[:, :], AX.X, OP.add)
            nc.scalar.activation(dsc[:, 1:2], dsc[:, 0:1], AF.Sigmoid)
            nc.vector.tensor_scalar_mul(dsc[:, 2:3], dsc[:, 1:2], float(MD))
            nc.vector.reciprocal(dsc[:, 3:4], dsc[:, 2:3])

            # w' = exp(-dist/d)/sum (+1 center)
            psum_i11 = psb.tile([11, 1], F32, tag="psum_i11")
            nc.tensor.matmul(psum_i11[:, :], o111_v, dsc[:, 3:4],
                             start=True, stop=True)
            invd_col = pool.tile([11, 1], F32, tag="invd_col")
            nc.vector.tensor_copy(invd_col[:, :], psum_i11[:, :])
            wexp = pool.tile([11, 11], F32, tag="wexp")
            nc.scalar.activation(wexp[:, :], dnt_v, AF.Exp,
                                 scale=invd_col[:, 0:1])
            psum_ws = psb.tile([1, 11], F32, tag="psum_ws")
            nc.tensor.matmul(psum_ws[:, :], o11_v, wexp[:, :],
                             start=True, stop=True)
            wsv = pool.tile([1, 2], F32, tag="wsv")
            nc.vector.tensor_reduce(wsv[:, 0:1], psum_ws[:, :], AX.X, OP.add)
            nc.vector.reciprocal(wsv[:, 1:2], wsv[:, 0:1])
            psum_w11 = psb.tile([11, 1], F32, tag="psum_w11")
            nc.tensor.matmul(psum_w11[:, :], o111_v, wsv[:, 1:2],
                             start=True, stop=True)
            wsi_col = pool.tile([11, 1], F32, tag="wsi_col")
            nc.vector.tensor_copy(wsi_col[:, :], psum_w11[:, :])
            wp_sb = pool.tile([11, 400], F32, tag="wp_sb")
            nc.vector.memset(wp_sb[:, :], 0.0)
            nc.vector.tensor_scalar_mul(wp_sb[:, 250:261], wexp[:, :],
                                        wsi_col[:, 0:1])
            nc.sync.dma_start(out=vdram[:, :], in_=wp_sb[:, :])

            # T matrices via sliding-window DMA + fold
            t_ext = pool.tile([128, 11, 138], F32, tag="t_ext")
            # Load T with all-positive steps (contiguous 552B runs) by
            # storing j REVERSED: t_ext[p, dy, jr] = v[dy, 123 + p + jr]
            # (valid because the kernel rows are symmetric in dx, so the
            # reversed generator equals the original). xe = 132 - jr.
            nc.gpsimd.dma_start(
                out=t_ext[:, :, :],
                in_=AP(tensor=vdram, offset=123,
                       ap=[[1, 128], [400, 11], [1, 138]]))
            # folds in reversed coords: xe=0 target at jr=132 (sources
            # jr 133..137), xe=127 target at jr=5 (sources jr 0..4)
            tl = pool.tile([128, 11, 1], F32, tag="tl")
            th = pool.tile([128, 11, 1], F32, tag="th")
            nc.vector.tensor_reduce(tl[:, :, :], t_ext[:, :, 133:138], AX.X, OP.add)
            nc.vector.tensor_reduce(th[:, :, :], t_ext[:, :, 0:5], AX.X, OP.add)
            nc.vector.tensor_tensor(t_ext[:, :, 132:133], t_ext[:, :, 132:133],
                                    tl[:, :, :], OP.add)
            nc.vector.tensor_tensor(t_ext[:, :, 5:6], t_ext[:, :, 5:6],
                                    th[:, :, :], OP.add)
            # t8[p, r, xout] = T_{dy=5-r}[p, xout] in fp8e4 (reversed dy order
            # so the DoubleRow rhs k-tile step is +64); slot r=11 is zero pad.
            t8 = pool.tile([128, 12, 128], mybir.dt.float8e4, tag="t8")
            nc.vector.memset(t8[:, 11, :], 0.0)
            tea = t_ext[:, :, :]
            nc.vector.tensor_copy(
                t8[:, 0:11, :],
                AP(tensor=tea.tensor, offset=tea.offset + 10 * 138 + 132,
                   ap=[list(tea.ap[0]), [-138, 11], [-1, 128]]))

            # ============== P merge (after scatter chains land) ==============
            def pslot_dst(buf, t0):
                # [128, 64 slots step 2, 64] view into buf starting at slot t0
                a = buf[:, :]
                return AP(tensor=a.tensor, offset=a.offset + t0 * 64,
                          ap=[list(a.ap[0]), [128, 64], [1, 64]])

            for par in range(2):
                ev = pbuf[par][:, 0:4096].rearrange("p (a b) -> p a b", b=64)
                od = pbuf[2 + par][:, 0:4096].rearrange("p (a b) -> p a b", b=64)
                nc.vector.tensor_tensor(pslot_dst(p8, PT_OFF + par), ev, od,
                                        OP.add)
                nc.vector.tensor_tensor(pslot_dst(p_t, PT_OFF + par), ev, od,
                                        OP.add)

            # ============== Phase C: Toeplitz conv + tail ==============
            psum_stack.close()
            psum_stack = ExitStack()
            psc = psum_stack.enter_context(
                tc.tile_pool(name="psc", bufs=2, space="PSUM"))
            ft = pool.tile([128, 8192], BF16, tag="ft")

            p_t_flat = p_t[:, :]
            p8_flat = p8[:, :]
            DR = mybir.MatmulPerfMode.DoubleRow
            for ps in range(5):
                nslots = 32 if ps < 4 else 16
                psum_c = psc.tile([128, nslots * 64], F32, tag="psum_c")
                # fp8 DoubleRow Toeplitz: 6 dy pairs per 8-slot chunk
                # (PSUM writes are limited to one 512-f32 bank per matmul)
                for ch in range(nslots // 8):
                    u0 = 32 * ps + 8 * ch
                    for pi, r in enumerate(range(0, 12, 2)):
                        rhs = AP(tensor=p8_flat.tensor,
                                 offset=p8_flat.offset + (u0 + 3 + r) * 64,
                                 ap=[list(p8_flat.ap[0]), [64, 2], [1, 512]])
                        nc.tensor.matmul(
                            psum_c[:, ch * 512:ch * 512 + 512],
                            t8[:, r:r + 2, :], rhs,
                            start=(pi == 0), stop=False,
                            perf_mode=DR, skip_group_check=True)
                    # exact +P via bf16 identity (center +1 not in t8)
                    nc.tensor.matmul(
                        psum_c[:, ch * 512:ch * 512 + 512], ident_bf[:, :],
                        p_t_flat[:, (u0 + 8) * 64:(u0 + 8) * 64 + 512],
                        start=False, stop=True, skip_group_check=True)
                # tail: feat_T += psum slice  (real y rows of this pass)
                y0 = max(0, 32 * ps - 8)
                y1 = min(128, 32 * ps + 24)
                nc.sync.dma_start(out=ft[:, y0 * 64:y1 * 64],
                                  in_=feat_T[:, y0 * 64:y1 * 64])
                po = (y0 + 8 - 32 * ps) * 64
                nc.vector.tensor_tensor(
                    ft[:, y0 * 64:y1 * 64],
                    psum_c[:, po:po + (y1 - y0) * 64],
                    ft[:, y0 * 64:y1 * 64], OP.add)
                if ps == 0:
                    pa = psum_c[:, :]
                    tmpf = pool.tile([128, 64], F32, tag="tmpf")
                    nc.vector.tensor_reduce(
                        tmpf[:, :],
                        AP(tensor=pa.tensor, offset=pa.offset + 3 * 64,
                           ap=[list(pa.ap[0]), [1, 64], [64, 5]]),
                        AX.X, OP.add)
                    nc.vector.tensor_tensor(ft[:, 0:64], ft[:, 0:64],
                                            tmpf[:, :], OP.add)
                if ps == 4:
                    pa = psum_c[:, :]
                    tmph = pool.tile([128, 64], F32, tag="tmph")
                    nc.vector.tensor_reduce(
                        tmph[:, :],
                        AP(tensor=pa.tensor, offset=pa.offset + 8 * 64,
                           ap=[list(pa.ap[0]), [1, 64], [64, 5]]),
                        AX.X, OP.add)
                    nc.vector.tensor_tensor(ft[:, 127 * 64:128 * 64],
                                            ft[:, 127 * 64:128 * 64],
                                            tmph[:, :], OP.add)
                nc.sync.dma_start(out=out_T[:, y0 * 64:y1 * 64],
                                  in_=ft[:, y0 * 64:y1 * 64])

            psum_stack.close()

    nc.compile()
    return nc


def build_core_inputs(x, xyz, feature, conv_w, conv_b, gn_gamma, gn_beta,
                      mlp_w1, mlp_w2):
    """Host-side sharding glue: slice batch b per core + layout transforms."""
    import ml_dtypes
    f32 = np.float32
    bf16 = ml_dtypes.bfloat16
    # shared constants
    convw_pl = np.zeros((128, 4, 128), f32)
    sfw_pl = np.zeros((128, 4, 8), f32)
    for h in range(2):
        for g in range(4):
            convw_pl[64 * h:64 * h + 64, g, np.arange(16) * 8 + h * 4 + g] = conv_w.T
            sfw_pl[64 * h:64 * h + 64, g, h * 4 + g] = 1.0
    gmat = (np.arange(128)[:, None] // 32 == np.arange(4)[None, :]).astype(f32)
    ltri_m = (np.arange(128)[:, None] < np.arange(128)[None, :]).astype(f32)
    trash_m = (16384 + (np.arange(32)[None, :] % 2) * 128
               + np.arange(128)[:, None]).astype(f32)
    ident_m = np.eye(128, dtype=f32)
    selw_m = np.zeros((128, 128), f32)
    for g in range(8):
        for q in range(16):
            selw_m[16 * g + q, g * 16 + q] = 1.0
    blob1 = np.zeros((128, 968), f32)
    blob1[:, 0:128] = ltri_m
    blob1[:, 128:256] = ident_m
    blob1[:, 256:384] = selw_m
    blob1[:, 384:896] = convw_pl.reshape(128, 512)
    blob1[:, 896:928] = trash_m
    blob1[:, 928:960] = sfw_pl.reshape(128, 32)
    blob1[:, 960:964] = gmat
    blob1[:, 964] = np.repeat(conv_b, 8)
    blob1[:, 965] = np.repeat(gn_gamma, 8)
    blob1[:, 966] = np.repeat(gn_beta, 8)
    blob1[:, 967] = 1.0
    dxy = np.arange(11) - 5
    blob2 = np.zeros((11, 546), f32)
    blob2[0:4, 0:128] = gmat.T
    blob2[0:8, 128:256] = (np.arange(128)[None, :] % 8
                           == np.arange(8)[:, None]).astype(f32)
    blob2[0, 256:384] = mlp_w2[0]
    blob2[0, 384:512] = 1.0
    blob2[0:11, 512:523] = -np.sqrt(dxy[None, :] ** 2 + dxy[:, None] ** 2)
    blob2[5, 523 + 5] = 1.0
    blob2[0:11, 534] = 1.0
    blob2[0, 535:546] = 1.0
    # w1 rearranged: w1n[p=(o,seg), j=(by_l,bx), n] = mlp_w1[n, o*256+(seg*2+by_l)*16+bx]
    o = np.arange(16)[:, None, None, None]
    seg = np.arange(8)[None, :, None, None]
    byl = np.arange(2)[None, None, :, None]
    bx = np.arange(16)[None, None, None, :]
    fl = (o * 256 + (seg * 2 + byl) * 16 + bx).reshape(128, 32)
    w1n = np.ascontiguousarray(mlp_w1.T[fl]).astype(bf16)  # [128, 32, 128]
    shared = dict(blob1=blob1, blob2=blob2, w1n=w1n)

    in_maps = []
    for b in range(B):
        fb = np.ascontiguousarray(feature[b].reshape(64, 16384)).astype(bf16)
        m = dict(shared)
        m["x_wrap"] = np.ascontiguousarray(
            x[b].reshape(32, 128, 64).transpose(1, 0, 2)).astype(bf16)
        m["xx_w2"] = np.ascontiguousarray(xyz[b, :, 0].reshape(32, 128).T).astype(f32)
        m["xy_w2"] = np.ascontiguousarray(xyz[b, :, 1].reshape(32, 128).T).astype(f32)
        m["feat_nat2"] = np.concatenate([fb[:, :8192], fb[:, 8192:]], axis=0)
        m["feat_T"] = np.ascontiguousarray(
            feature[b].transpose(2, 1, 0).reshape(128, 8192)).astype(bf16)
        in_maps.append(m)
    return in_maps


_NC_CACHE = {}


def kernel(x, xyz, feature, conv_w, conv_b, gn_gamma, gn_beta, mlp_w1, mlp_w2,
           _trace=False):
    from concourse.bass_utils import run_bass_kernel_spmd
    if "nc" not in _NC_CACHE:
        _NC_CACHE["nc"] = build_nc()
    nc = _NC_CACHE["nc"]
    in_maps = build_core_inputs(np.asarray(x), np.asarray(xyz),
                                np.asarray(feature), np.asarray(conv_w),
                                np.asarray(conv_b), np.asarray(gn_gamma),
                                np.asarray(gn_beta), np.asarray(mlp_w1),
                                np.asarray(mlp_w2))
    res = run_bass_kernel_spmd(nc, in_maps, core_ids=list(range(8)),
                               trace=_trace)
    outs = []
    for i in range(B):
        ot = np.asarray(res.results[i]["out_T"]).astype(np.float32)
        outs.append(ot.reshape(128, 128, 64).transpose(2, 1, 0))
    out = np.stack(outs).astype(np.float32)
    if _trace:
        return out, res
    return out
